# revision 29
# baseline (speedup 1.0000x reference)
"""Trainium2 Bass kernel for nn_BinarizedCifar10MLP.

Data-parallel over batch (8192/8 = 1024 rows/core), feature-major activation
layout [features, batch] on device.  BN batch statistics are all-reduced in two
chunks per layer so the collective latency hides under the layer's tail
matmuls; the next layer's matmuls are emitted k-phased (k < SPLIT*128 first for
the first 8 PSUM groups) so they start before the late stats chunk lands.

Precision scheme (reference is fp32, gate rel_err < 2e-2):
  - Weights are sign(+-1), pre-signed on the host: W1 as fp16 (exact), W2/W3 as
    fp8e4m3 (exact +-1) driven in DoubleRow mode at 2x PE rate.
  - L1 (x @ sign(W1).T): x split on host into fp16 hi + lo pieces.  lo either
    exact fp16 (L1MODE=hi16lo16) or fp8e5m2 of lo*2^11 matched with
    sign(W1)*2^-11 fp8e5m2 weights in DoubleRow mode (L1MODE=hi16lo8), which
    accumulates into the same fp32 PSUM group as hi.
  - L2/L3: +-1 x +-1 products accumulate exactly in fp32 PSUM.
  - L4: y3/W4 in fp16, log-softmax in fp32.
"""

import sys

sys.path.insert(0, "/opt/trn_rl_repo")

import numpy as np
import ml_dtypes

B, D, H, C = 8192, 3 * 32 * 32, 2048, 10
EPS = 1e-5
NCORES = 8
BS = B // NCORES          # batch rows per core
KD = D // 128             # 24 k-tiles over input dim
KH = H // 128             # 16 k-tiles over hidden dim
NB = BS // 512            # 2 free-dim chunks of 512
SPLIT = 12                # m-tiles covered by the early stats AllReduce chunk
P1G = 8                   # psum groups in the k-phased prologue of L2/L3
XCH = 8                   # x DMA chunks
ACT_EVERY = 4             # every ACT_EVERY-th sign/y3 tile goes to ScalarE

L1MODE = "hi16lo8"        # "hi16lo16" (exact) | "hi16lo8" (fp8 DoubleRow lo)

_CACHE = {}


def _build(stage=7, fast=(False, False), l1mode=None):
    import concourse.bacc as bacc
    import concourse.mybir as mybir
    import concourse.tile as tile

    l1mode = l1mode or L1MODE
    lo8 = l1mode == "hi16lo8"
    F32 = mybir.dt.float32
    F16 = mybir.dt.float16
    F8E4 = mybir.dt.float8e4
    F8E5 = mybir.dt.float8e5
    DR = mybir.MatmulPerfMode.DoubleRow
    ACT = mybir.ActivationFunctionType
    ALU = mybir.AluOpType
    RG = [list(range(NCORES))]

    nc = bacc.Bacc("TRN2", target_bir_lowering=False, debug=False, num_devices=NCORES)

    # ---- I/O ----
    xhi_d = nc.dram_tensor("xT_hi", [D, BS], F16, kind="ExternalInput").ap()
    if lo8:
        xlo_d = nc.dram_tensor("xT_lo8", [D, BS], F8E5, kind="ExternalInput").ap()
        w1lo_d = nc.dram_tensor("w1lopk", [128, KH * KD * 128], F8E5, kind="ExternalInput").ap()
    else:
        xlo_d = nc.dram_tensor("xT_lo", [D, BS], F16, kind="ExternalInput").ap()
        w1lo_d = None
    w1pk_d = nc.dram_tensor("w1pk", [128, KH * KD * 128], F16, kind="ExternalInput").ap()
    w2pk_d = nc.dram_tensor("w2pk", [128, KH * KH * 128], F8E4, kind="ExternalInput").ap()
    w3pk_d = nc.dram_tensor("w3pk", [128, KH * KH * 128], F8E4, kind="ExternalInput").ap()
    CNAMES = ("b1", "g1", "bt1", "b2", "g2", "bt2", "b3", "g3", "bt3")
    cpk_d = nc.dram_tensor("cpk", [128, KH * len(CNAMES)], F32, kind="ExternalInput").ap()
    w4pk_d = nc.dram_tensor("w4pk", [128, C * KH], F16, kind="ExternalInput").ap()
    b4_d = nc.dram_tensor("c_b4", [16, 1], F32, kind="ExternalInput").ap()
    out_d = nc.dram_tensor("outT", [C, BS], F32, kind="ExternalOutput").ap()
    wpk_d = {2: w2pk_d, 3: w3pk_d}

    with tile.TileContext(nc) as tc:
        with (
            tc.tile_pool(name="pconst", bufs=1) as pconst,
            tc.tile_pool(name="pstat", bufs=1) as pstat,
            tc.tile_pool(name="plog", bufs=1) as plog,
            tc.tile_pool(name="pscr", bufs=2) as pscr,
            tc.tile_pool(name="pw1", bufs=2) as pw1,
            tc.tile_pool(name="pw1lo", bufs=2) as pw1lo,
            tc.tile_pool(name="pw8", bufs=5) as pw8,
            tc.tile_pool(name="pa", bufs=1) as pa,
            tc.tile_pool(name="pb", bufs=1) as pb,
            tc.tile_pool(name="pa2", bufs=1) as pa2,
            tc.tile_pool(name="ph", bufs=1) as ph,
            tc.tile_pool(name="ppsum", bufs=8, space="PSUM") as ppsum,
            tc.tile_pool(name="pdram", bufs=1, space="DRAM") as pdram,
        ):
            # ---- L1 weight tiles (prefetch m=0,1 BEFORE x so the first
            # matmul's weights are not queued behind the whole x load) ----
            w1_tiles = {}

            def ensure_w1(m):
                if m not in w1_tiles:
                    w16 = pw1.tile([128, KD * 128], F16, tag="w1", name=f"w16_{m}")
                    nc.sync.dma_start(w16[:], w1pk_d[:, m * KD * 128:(m + 1) * KD * 128])
                    if lo8:
                        wlo = pw1lo.tile([128, KD * 128], F8E5, tag="w1lo", name=f"wlo_{m}")
                        nc.sync.dma_start(wlo[:], w1lo_d[:, m * KD * 128:(m + 1) * KD * 128])
                        w1_tiles[m] = (w16, wlo[:].rearrange("p (k c) -> p k c", c=128))
                    else:
                        w1_tiles[m] = (w16, None)
                return w1_tiles[m]

            ensure_w1(0)
            ensure_w1(1)

            # ---- load x pieces in chunks so the first matmuls start early ----
            xhi = pa.tile([128, KD * BS], F16, tag="pa", name="xhi")
            if lo8:
                xlo = pb.tile([128, KD * BS], F8E5, tag="pb", name="xlo")
            else:
                xlo = pb.tile([128, KD * BS], F16, tag="pb", name="xlo")
            xhiv = xhi[:].rearrange("p (k c) -> p k c", c=BS)
            xlov = xlo[:].rearrange("p (k c) -> p k c", c=BS)
            xhisrc = xhi_d.rearrange("(k p) c -> p k c", p=128)
            xlosrc = xlo_d.rearrange("(k p) c -> p k c", p=128)
            kpc = KD // XCH
            for ch in range(XCH):
                k0, k1 = ch * kpc, (ch + 1) * kpc
                nc.sync.dma_start(xhiv[:, k0:k1, :], xhisrc[:, k0:k1, :])
            for ch in range(XCH):
                k0, k1 = ch * kpc, (ch + 1) * kpc
                nc.sync.dma_start(xlov[:, k0:k1, :], xlosrc[:, k0:k1, :])

            # ---- constants ----
            cpk = pconst.tile([128, KH * len(CNAMES)], F32, tag="cpk")
            nc.sync.dma_start(cpk[:], cpk_d)
            cons = {name: cpk[:, i * KH:(i + 1) * KH] for i, name in enumerate(CNAMES)}

            # prefetch the L2 phase-1 weight tiles so they are resident at the
            # L1->L2 transition
            w8_tiles = {}

            def ensure_w8(l, m):
                if (l, m) not in w8_tiles:
                    w = pw8.tile([128, KH * 128], F8E4, tag="w8", name=f"w8_{l}_{m}")
                    nc.sync.dma_start(w[:], wpk_d[l][:, m * KH * 128:(m + 1) * KH * 128])
                    w8_tiles[(l, m)] = w[:].rearrange("p (k c) -> p k c", c=128)
                return w8_tiles[(l, m)]

            if stage >= 3:
                for m in range((P1G + NB - 1) // NB):
                    ensure_w8(2, m)

            b4s = pconst.tile([16, 1], F32, tag="b4")
            nc.sync.dma_start(b4s[:], b4_d)
            w4f = pconst.tile([128, C * KH], F16, tag="w4f")
            nc.sync.dma_start(w4f[:], w4pk_d)

            parts = {}
            stats = {}

            def st(l, tag):
                key = (l, tag)
                if key not in stats:
                    stats[key] = pstat.tile([128, KH], F32, name=f"{tag}{l}", tag=f"{tag}{l}")
                return stats[key]

            def is_fast(l):
                return l < 3 and fast[l - 1]

            def stats_chunk(l, m0, m1, g_t):
                """g_t: [128, 2d] (fast) or [128, 4d] (full): [sums | sqsums]."""
                d = m1 - m0
                red = pstat.tile([128, d], F32, tag=f"red{l}{m0}", name=f"red{l}{m0}")
                nc.vector.tensor_reduce(
                    red[:], g_t[:, 0:2 * d].rearrange("p (m n) -> p m n", n=2),
                    axis=mybir.AxisListType.X, op=ALU.add)
                if is_fast(l):
                    nc.vector.tensor_scalar_mul(st(l, "thr")[:, m0:m1], red[:], 1.0 / B)
                    return
                redq = pstat.tile([128, d], F32, tag=f"redq{l}{m0}", name=f"redq{l}{m0}")
                nc.vector.tensor_reduce(
                    redq[:], g_t[:, 2 * d:4 * d].rearrange("p (m n) -> p m n", n=2),
                    axis=mybir.AxisListType.X, op=ALU.add)
                sl = slice(m0, m1)
                m1c, msq, m1sq, v, sq, r, rp, mt, c = (
                    st(l, x) for x in ("m1", "msq", "m1sq", "v", "sq", "r", "rp", "mt", "c"))
                nc.vector.tensor_scalar_mul(m1c[:, sl], red[:], 1.0 / B)
                nc.vector.tensor_scalar_mul(msq[:, sl], redq[:], 1.0 / B)
                nc.vector.tensor_tensor(m1sq[:, sl], m1c[:, sl], m1c[:, sl], op=ALU.mult)
                nc.vector.tensor_tensor(v[:, sl], msq[:, sl], m1sq[:, sl], op=ALU.subtract)
                nc.vector.tensor_scalar_add(v[:, sl], v[:, sl], EPS)
                nc.scalar.activation(sq[:, sl], v[:, sl], ACT.Sqrt)
                nc.vector.reciprocal(r[:, sl], sq[:, sl])
                nc.vector.tensor_tensor(rp[:, sl], cons[f"g{l}"][:, sl], r[:, sl], op=ALU.mult)
                nc.vector.tensor_tensor(mt[:, sl], m1c[:, sl], rp[:, sl], op=ALU.mult)
                nc.vector.tensor_tensor(c[:, sl], cons[f"bt{l}"][:, sl], mt[:, sl], op=ALU.subtract)
                if l < 3:
                    # binarized threshold t = m - bt/(g*r); the +-sign(g) factor
                    # is folded into the next layer's weights/bias host-side
                    gi, u, u2, tthr = (st(l, x) for x in ("gi", "u", "u2", "tthr"))
                    nc.vector.reciprocal(gi[:, sl], cons[f"g{l}"][:, sl])
                    nc.vector.tensor_tensor(u[:, sl], cons[f"bt{l}"][:, sl], gi[:, sl], op=ALU.mult)
                    nc.vector.tensor_tensor(u2[:, sl], u[:, sl], sq[:, sl], op=ALU.mult)
                    nc.vector.tensor_tensor(tthr[:, sl], m1c[:, sl], u2[:, sl], op=ALU.subtract)

            def boundary(l, m0, m1, tag):
                """AllReduce parts cols for m-tiles [m0, m1) and compute stats."""
                d = m1 - m0
                w = 2 * d if is_fast(l) else 4 * d
                arin = pdram.tile([128, w], F32, tag=f"arin{l}{tag}")
                arout = pdram.tile([128, w], F32, tag=f"arout{l}{tag}")
                nc.sync.dma_start(arin[:, 0:2 * d], parts[l][:, 2 * m0:2 * m1])
                if not is_fast(l):
                    nc.sync.dma_start(arin[:, 2 * d:4 * d], parts[l][:, 32 + 2 * m0:32 + 2 * m1])
                nc.gpsimd.collective_compute(
                    "AllReduce", ALU.add, replica_groups=RG,
                    ins=[arin.opt()], outs=[arout.opt()])
                g_t = pstat.tile([128, w], F32, tag=f"g{l}{tag}", name=f"g{l}{tag}")
                nc.sync.dma_start(g_t[:], arout[:])
                stats_chunk(l, m0, m1, g_t)

            def sign_chunk(l, dst3, h_t, k0, k1, act_every=0):
                # {0,1}-coded activations: one is_ge per tile; the 2b-1
                # decode is folded into the next layer's weights (+-2) and bias
                fastl = is_fast(l)
                for k in range(k0, k1):
                    hsl = h_t[:, k * BS:(k + 1) * BS]
                    thr = st(l, "thr" if fastl else "tthr")[:, k:k + 1]
                    nc.vector.tensor_scalar(out=dst3[:, k, :], in0=hsl, scalar1=thr,
                                            scalar2=None, op0=ALU.is_ge)

            def y3_chunk(dst, h_t, k0, k1, act_every):
                rp3, c3 = st(3, "rp"), st(3, "c")
                for k in range(k0, k1):
                    hsl = h_t[:, k * BS:(k + 1) * BS]
                    scr = pscr.tile([128, BS], F32, tag="scr", name=f"y3s_{k}")
                    if act_every and (k % act_every == act_every - 1):
                        nc.scalar.activation(scr[:], hsl, ACT.Identity,
                                             bias=c3[:, k:k + 1], scale=rp3[:, k:k + 1])
                    else:
                        nc.vector.tensor_scalar(out=scr[:], in0=hsl, scalar1=rp3[:, k:k + 1],
                                                scalar2=c3[:, k:k + 1], op0=ALU.mult, op1=ALU.add)
                    nc.vector.tensor_scalar(out=dst[:, k * BS:(k + 1) * BS], in0=scr[:],
                                            scalar1=-1.0, scalar2=1.0, op0=ALU.max, op1=ALU.min)

            def finish_group(l, h_t, ps, m, n):
                hs = h_t[:, m * BS + n * 512: m * BS + n * 512 + 512]
                col = 2 * m + n
                nc.scalar.activation(hs, ps[:], ACT.Identity, bias=cons[f"b{l}"][:, m:m + 1],
                                     scale=1.0, accum_out=parts[l][:, col:col + 1])
                if not is_fast(l):
                    scr = pscr.tile([128, BS], F32, tag="scr", name=f"sq_{l}_{m}_{n}")
                    nc.scalar.activation(scr[:, :512], hs, ACT.Square,
                                         accum_out=parts[l][:, 32 + col:32 + col + 1])

            # ===================== Layer 1 =====================
            h1 = ph.tile([128, KH * BS], F32, tag="ph", name="h1")
            parts[1] = pstat.tile([128, 64], F32, tag="parts1", name="parts1")
            a2 = pa2.tile([128, KH, BS], F8E4, tag="pa2", name="a2")
            for m in range(KH):
                w16, wlov = ensure_w1(m)
                if m + 2 < KH:
                    ensure_w1(m + 2)
                for n in range(NB):
                    ps = ppsum.tile([128, 512], F32, tag="ps", name=f"ps_1_{m}_{n}")
                    if lo8:
                        for k in range(KD):
                            nc.tensor.matmul(ps[:], w16[:, k * 128:(k + 1) * 128],
                                             xhi[:, k * BS + n * 512: k * BS + n * 512 + 512],
                                             start=(k == 0), stop=False)
                        for t in range(KD // 2):
                            nc.tensor.matmul(ps[:], wlov[:, 2 * t:2 * t + 2, :],
                                             xlov[:, 2 * t:2 * t + 2, n * 512:n * 512 + 512],
                                             start=False, stop=(t == KD // 2 - 1), perf_mode=DR)
                    else:
                        for k in range(KD):
                            lhsT = w16[:, k * 128:(k + 1) * 128]
                            sl = slice(k * BS + n * 512, k * BS + n * 512 + 512)
                            nc.tensor.matmul(ps[:], lhsT, xhi[:, sl], start=(k == 0), stop=False)
                            nc.tensor.matmul(ps[:], lhsT, xlo[:, sl], start=False, stop=(k == KD - 1))
                    finish_group(1, h1, ps, m, n)
                if m == SPLIT - 1 and stage >= 2:
                    boundary(1, 0, SPLIT, "A")
                    if lo8:
                        # sign-wave A overlaps the L1 tail (a2 has its own slot)
                        sign_chunk(1, a2, h1, 0, SPLIT)
            if stage == 1:
                nc.sync.dma_start(out_d[:], h1[:C, :BS])
            if stage >= 2:
                boundary(1, SPLIT, KH, "B")
                if not lo8:
                    sign_chunk(1, a2, h1, 0, SPLIT)
                sign_chunk(1, a2, h1, SPLIT, KH)
                if stage == 2:
                    t = pscr.tile([128, BS], F32, tag="scr", name="dbg2")
                    nc.vector.tensor_scalar(out=t[:C, :], in0=a2[:C, 0, :], scalar1=2.0,
                                            scalar2=-1.0, op0=ALU.mult, op1=ALU.add)
                    nc.sync.dma_start(out_d[:], t[:C, :])

            # ================= Layers 2 and 3 (DoubleRow fp8) =================
            # stats chunk plans: early chunks fire mid-layer so the collective
            # rendezvous+exec hides under tail matmuls; the CC runs them FIFO
            CHUNK_PLAN = {2: [(0, SPLIT)], 3: [(0, 8), (8, SPLIT)]}
            FINAL_C0 = {2: SPLIT, 3: SPLIT}
            TSPL = {2: SPLIT // 2, 3: SPLIT // 2}  # phase-1 t-pairs

            def dense_dr(l, rhs3, dst3_or_y3):
                h_t = ph.tile([128, KH * BS], F32, tag="ph", name=f"h{l}")
                parts[l] = pstat.tile([128, 64], F32, tag=f"parts{l}", name=f"parts{l}")
                groups = [(m, n) for m in range(KH) for n in range(NB)]
                psums = {}

                def emit(g, t0, t1):
                    m, n = groups[g]
                    wv = ensure_w8(l, m)
                    if g not in psums:
                        psums[g] = ppsum.tile([128, 512], F32, tag="ps", name=f"ps_{l}_{g}")
                    ps = psums[g]
                    for t in range(t0, t1):
                        nc.tensor.matmul(ps[:], wv[:, 2 * t:2 * t + 2, :],
                                         rhs3[:, 2 * t:2 * t + 2, n * 512:n * 512 + 512],
                                         start=(t == 0), stop=(t == KH // 2 - 1), perf_mode=DR)

                TSPLIT = TSPL[l]
                for g in range(P1G):
                    emit(g, 0, TSPLIT)
                for g in range(P1G):
                    emit(g, TSPLIT, KH // 2)
                    finish_group(l, h_t, psums[g], *groups[g])
                for g in range(P1G, len(groups)):
                    emit(g, 0, KH // 2)
                    finish_group(l, h_t, psums[g], *groups[g])
                    m, n = groups[g]
                    for c0, c1 in CHUNK_PLAN[l]:
                        if (m, n) == (c1 - 1, NB - 1):
                            boundary(l, c0, c1, f"c{c0}")
                            if l < 3:
                                sign_chunk(l, dst3_or_y3, h_t, c0, c1)
                            else:
                                y3_chunk(dst3_or_y3, h_t, c0, c1, act_every=1)
                    if l == 2 and stage >= 4 and (m, n) == (KH - 1, 0):
                        # all L2 weight DMAs are emitted; now prefetch L3 phase-1
                        for mp in range((P1G + NB - 1) // NB):
                            ensure_w8(3, mp)
                c0 = FINAL_C0[l]
                boundary(l, c0, KH, "Z")
                if l < 3:
                    sign_chunk(l, dst3_or_y3, h_t, c0, KH)
                else:
                    y3_chunk(dst3_or_y3, h_t, c0, KH, act_every=2)
                return h_t

            if stage >= 3:
                a3 = pa.tile([128, KH, BS], F8E4, tag="pa", name="a3")
                dense_dr(2, a2[:], a3)
                if stage == 3:
                    t = pscr.tile([128, BS], F32, tag="scr", name="dbg3")
                    nc.vector.tensor_scalar(out=t[:C, :], in0=a3[:C, 0, :], scalar1=2.0,
                                            scalar2=-1.0, op0=ALU.mult, op1=ALU.add)
                    nc.sync.dma_start(out_d[:], t[:C, :])

            if stage >= 4:
                y3 = pb.tile([128, KH * BS], F16, tag="pb", name="y3")
                dense_dr(3, a3[:], y3)
                if stage == 4:
                    t = pscr.tile([128, BS], F32, tag="scr", name="dbg4")
                    nc.vector.tensor_copy(t[:C, :], y3[:C, :BS])
                    nc.sync.dma_start(out_d[:], t[:C, :])

            if stage >= 5:
                # ===== Layer 4: logits out (log-softmax happens on the host) =====
                logits = plog.tile([16, BS], F32, tag="logits")
                ps4s = {}
                for n in range(NB):
                    ps4s[n] = ppsum.tile([128, 512], F32, tag="ps", name=f"ps4_{n}")
                    for k in range(SPLIT):
                        nc.tensor.matmul(ps4s[n][:C, :], w4f[:, k * C:(k + 1) * C],
                                         y3[:, k * BS + n * 512: k * BS + n * 512 + 512],
                                         start=(k == 0), stop=False)
                for n in range(NB):
                    for k in range(SPLIT, KH):
                        nc.tensor.matmul(ps4s[n][:C, :], w4f[:, k * C:(k + 1) * C],
                                         y3[:, k * BS + n * 512: k * BS + n * 512 + 512],
                                         start=False, stop=(k == KH - 1))
                    nc.scalar.activation(logits[:C, n * 512:(n + 1) * 512], ps4s[n][:C, :],
                                         ACT.Identity, bias=b4s[:C, :], scale=1.0)
                    nc.sync.dma_start(out_d[:, n * 512:(n + 1) * 512],
                                      logits[:C, n * 512:(n + 1) * 512])

    nc.compile()
    return nc


def _sgn(a):
    return np.where(np.asarray(a, np.float32) >= 0, np.float32(1), np.float32(-1))


def _pack(S):
    """[M*128, K*128] -> [128, M*K*128] with w[p, (m*K+k)*128+c] = S[m*128+c, k*128+p]."""
    M, K = S.shape[0] // 128, S.shape[1] // 128
    A = S.reshape(M, 128, K, 128)  # [m, c, k, p]
    return np.ascontiguousarray(A.transpose(3, 0, 2, 1).reshape(128, M * K * 128))


def _prep_inputs(x, W1, b1, g1, bt1, W2, b2, g2, bt2, W3, b3, g3, bt3, W4, b4,
                 l1mode=None):
    """Host-side sharding + layout prep (pure layout/sign/lossless-split work)."""
    l1mode = l1mode or L1MODE

    def as32(a):
        return np.ascontiguousarray(np.asarray(a, dtype=np.float32))

    x = as32(x)
    s1 = _pack(_sgn(W1))
    # {0,1}-coded activations: the device computes b = 1{h >= t}; the true
    # activation is a = s*(2b-1) with s = sign(gamma) of the producing layer.
    # Fold: h' = sum_d b_d * (2 s_d sW[f,d]) + (bias[f] - sum_d s_d sW[f,d]).
    s1mul = _sgn(g1)
    s2mul = _sgn(g2)
    S2 = _sgn(W2)
    S3 = _sgn(W3)
    shared = {
        "w1pk": s1.astype(np.float16),
        "w2pk": _pack(2.0 * S2 * s1mul[None, :]).astype(ml_dtypes.float8_e4m3fn),
        "w3pk": _pack(2.0 * S3 * s2mul[None, :]).astype(ml_dtypes.float8_e4m3fn),
    }
    if l1mode == "hi16lo8":
        shared["w1lopk"] = (s1 * 2.0 ** -11).astype(ml_dtypes.float8_e5m2)
    b2eff = as32(b2).reshape(-1) - S2 @ s1mul
    b3eff = as32(b3).reshape(-1) - S3 @ s2mul
    cvecs = (b1, g1, bt1, b2eff, g2, bt2, b3eff, g3, bt3)
    cpk = np.empty((128, KH * len(cvecs)), np.float32)
    for i, v in enumerate(cvecs):
        cpk[:, i * KH:(i + 1) * KH] = as32(v).reshape(KH, 128).T
    shared["cpk"] = cpk
    w4T = np.ascontiguousarray(as32(W4).T)          # [H, C]
    w4pk = np.empty((128, C * KH), np.float16)
    for k in range(KH):
        w4pk[:, k * C:(k + 1) * C] = w4T[k * 128:(k + 1) * 128, :].astype(np.float16)
    shared["w4pk"] = w4pk
    b4p = np.zeros((16, 1), np.float32)
    b4p[:C, 0] = as32(b4).reshape(-1)
    shared["c_b4"] = b4p

    in_maps = []
    for cidx in range(NCORES):
        xT = np.ascontiguousarray(x[cidx * BS:(cidx + 1) * BS].T)     # [D, BS]
        hi = xT.astype(np.float16)
        lo = xT - hi.astype(np.float32)
        m = dict(shared)
        m["xT_hi"] = hi
        if l1mode == "hi16lo8":
            m["xT_lo8"] = (lo * 2048.0).astype(ml_dtypes.float8_e5m2)
        else:
            m["xT_lo"] = lo.astype(np.float16)
        in_maps.append(m)
    return in_maps


def _fast_flags(inputs):
    """Mean-only BN boundaries are valid when beta==0 and gamma>0."""
    def ok(g, bt):
        g, bt = np.asarray(g), np.asarray(bt)
        return bool(not np.any(bt) and np.all(g > 0))

    return (ok(inputs["g1"], inputs["bt1"]), ok(inputs["g2"], inputs["bt2"]))


def _gather(res):
    out = np.concatenate([res.results[c]["outT"].T for c in range(NCORES)], axis=0)
    return out.astype(np.float32)


def _log_softmax(lg):
    mx = lg.max(axis=1, keepdims=True)
    return lg - mx - np.log(np.exp(lg - mx).sum(axis=1, keepdims=True))


def kernel(**inputs) -> np.ndarray:
    from concourse.bass_utils import run_bass_kernel_spmd

    fast = _fast_flags(inputs)
    if _CACHE.get("key") != (fast, L1MODE):
        _CACHE["nc"] = _build(fast=fast)
        _CACHE["key"] = (fast, L1MODE)
    nc = _CACHE["nc"]
    in_maps = _prep_inputs(**inputs)
    res = run_bass_kernel_spmd(nc, in_maps, list(range(NCORES)))
    return _log_softmax(_gather(res))


# revision 32
# speedup vs baseline: 1.0099x; 1.0099x over previous
"""Trainium2 Bass kernel for nn_BinarizedCifar10MLP.

Data-parallel over batch (8192/8 = 1024 rows/core), feature-major activation
layout [features, batch] on device.  BN batch statistics are all-reduced in two
chunks per layer so the collective latency hides under the layer's tail
matmuls; the next layer's matmuls are emitted k-phased (k < SPLIT*128 first for
the first 8 PSUM groups) so they start before the late stats chunk lands.

Precision scheme (reference is fp32, gate rel_err < 2e-2):
  - Weights are sign(+-1), pre-signed on the host: W1 as fp16 (exact), W2/W3 as
    fp8e4m3 (exact +-1) driven in DoubleRow mode at 2x PE rate.
  - L1 (x @ sign(W1).T): x split on host into fp16 hi + lo pieces.  lo either
    exact fp16 (L1MODE=hi16lo16) or fp8e5m2 of lo*2^11 matched with
    sign(W1)*2^-11 fp8e5m2 weights in DoubleRow mode (L1MODE=hi16lo8), which
    accumulates into the same fp32 PSUM group as hi.
  - L2/L3: +-1 x +-1 products accumulate exactly in fp32 PSUM.
  - L4: y3/W4 in fp16, log-softmax in fp32.
"""

import sys

sys.path.insert(0, "/opt/trn_rl_repo")

import numpy as np
import ml_dtypes

B, D, H, C = 8192, 3 * 32 * 32, 2048, 10
EPS = 1e-5
NCORES = 8
BS = B // NCORES          # batch rows per core
KD = D // 128             # 24 k-tiles over input dim
KH = H // 128             # 16 k-tiles over hidden dim
NB = BS // 512            # 2 free-dim chunks of 512
SPLIT = 12                # m-tiles covered by the early stats AllReduce chunk
P1G = 8                   # psum groups in the k-phased prologue of L2/L3
XCH = 8                   # x DMA chunks
ACT_EVERY = 4             # every ACT_EVERY-th sign/y3 tile goes to ScalarE

L1MODE = "hi16lo8"        # "hi16lo16" (exact) | "hi16lo8" (fp8 DoubleRow lo)

_CACHE = {}


def _build(stage=7, fast=(False, False), l1mode=None):
    import concourse.bacc as bacc
    import concourse.mybir as mybir
    import concourse.tile as tile

    l1mode = l1mode or L1MODE
    lo8 = l1mode == "hi16lo8"
    F32 = mybir.dt.float32
    F16 = mybir.dt.float16
    F8E4 = mybir.dt.float8e4
    F8E5 = mybir.dt.float8e5
    DR = mybir.MatmulPerfMode.DoubleRow
    ACT = mybir.ActivationFunctionType
    ALU = mybir.AluOpType
    RG = [list(range(NCORES))]

    nc = bacc.Bacc("TRN2", target_bir_lowering=False, debug=False, num_devices=NCORES)

    # ---- I/O ----
    xhi_d = nc.dram_tensor("xT_hi", [D, BS], F16, kind="ExternalInput").ap()
    if lo8:
        xlo_d = nc.dram_tensor("xT_lo8", [D, BS], F8E5, kind="ExternalInput").ap()
        w1lo_d = nc.dram_tensor("w1lopk", [128, KH * KD * 128], F8E5, kind="ExternalInput").ap()
    else:
        xlo_d = nc.dram_tensor("xT_lo", [D, BS], F16, kind="ExternalInput").ap()
        w1lo_d = None
    w1pk_d = nc.dram_tensor("w1pk", [128, KH * KD * 128], F16, kind="ExternalInput").ap()
    w2pk_d = nc.dram_tensor("w2pk", [128, KH * KH * 128], F8E4, kind="ExternalInput").ap()
    w3pk_d = nc.dram_tensor("w3pk", [128, KH * KH * 128], F8E4, kind="ExternalInput").ap()
    CNAMES = ("b1", "g1", "bt1", "b2", "g2", "bt2", "b3", "g3", "bt3")
    cpk_d = nc.dram_tensor("cpk", [128, KH * len(CNAMES)], F32, kind="ExternalInput").ap()
    w4pk_d = nc.dram_tensor("w4pk", [128, C * KH], F16, kind="ExternalInput").ap()
    b4_d = nc.dram_tensor("c_b4", [16, 1], F32, kind="ExternalInput").ap()
    out_d = nc.dram_tensor("outT", [C, BS], F32, kind="ExternalOutput").ap()
    wpk_d = {2: w2pk_d, 3: w3pk_d}

    with tile.TileContext(nc) as tc:
        with (
            tc.tile_pool(name="pconst", bufs=1) as pconst,
            tc.tile_pool(name="pstat", bufs=1) as pstat,
            tc.tile_pool(name="plog", bufs=1) as plog,
            tc.tile_pool(name="pscr", bufs=2) as pscr,
            tc.tile_pool(name="pw1", bufs=2) as pw1,
            tc.tile_pool(name="pw1lo", bufs=2) as pw1lo,
            tc.tile_pool(name="pw8", bufs=5) as pw8,
            tc.tile_pool(name="pa", bufs=1) as pa,
            tc.tile_pool(name="pb", bufs=1) as pb,
            tc.tile_pool(name="pa2", bufs=1) as pa2,
            tc.tile_pool(name="ph", bufs=1) as ph,
            tc.tile_pool(name="ppsum", bufs=8, space="PSUM") as ppsum,
            tc.tile_pool(name="pdram", bufs=1, space="DRAM") as pdram,
        ):
            # ---- L1 weight tiles (prefetch m=0,1 BEFORE x so the first
            # matmul's weights are not queued behind the whole x load) ----
            w1_tiles = {}

            def ensure_w1(m):
                if m not in w1_tiles:
                    w16 = pw1.tile([128, KD * 128], F16, tag="w1", name=f"w16_{m}")
                    nc.sync.dma_start(w16[:], w1pk_d[:, m * KD * 128:(m + 1) * KD * 128])
                    if lo8:
                        wlo = pw1lo.tile([128, KD * 128], F8E5, tag="w1lo", name=f"wlo_{m}")
                        nc.sync.dma_start(wlo[:], w1lo_d[:, m * KD * 128:(m + 1) * KD * 128])
                        w1_tiles[m] = (w16, wlo[:].rearrange("p (k c) -> p k c", c=128))
                    else:
                        w1_tiles[m] = (w16, None)
                return w1_tiles[m]

            ensure_w1(0)
            ensure_w1(1)

            # ---- load x pieces in chunks so the first matmuls start early ----
            xhi = pa.tile([128, KD * BS], F16, tag="pa", name="xhi")
            if lo8:
                xlo = pb.tile([128, KD * BS], F8E5, tag="pb", name="xlo")
            else:
                xlo = pb.tile([128, KD * BS], F16, tag="pb", name="xlo")
            xhiv = xhi[:].rearrange("p (k c) -> p k c", c=BS)
            xlov = xlo[:].rearrange("p (k c) -> p k c", c=BS)
            xhisrc = xhi_d.rearrange("(k p) c -> p k c", p=128)
            xlosrc = xlo_d.rearrange("(k p) c -> p k c", p=128)
            kpc = KD // XCH
            for ch in range(XCH):
                k0, k1 = ch * kpc, (ch + 1) * kpc
                nc.sync.dma_start(xhiv[:, k0:k1, :], xhisrc[:, k0:k1, :])
            for ch in range(XCH):
                k0, k1 = ch * kpc, (ch + 1) * kpc
                nc.sync.dma_start(xlov[:, k0:k1, :], xlosrc[:, k0:k1, :])

            # ---- constants ----
            cpk = pconst.tile([128, KH * len(CNAMES)], F32, tag="cpk")
            nc.sync.dma_start(cpk[:], cpk_d)
            cons = {name: cpk[:, i * KH:(i + 1) * KH] for i, name in enumerate(CNAMES)}

            # prefetch the L2 phase-1 weight tiles so they are resident at the
            # L1->L2 transition
            w8_tiles = {}

            def ensure_w8(l, m):
                if (l, m) not in w8_tiles:
                    w = pw8.tile([128, KH * 128], F8E4, tag="w8", name=f"w8_{l}_{m}")
                    nc.sync.dma_start(w[:], wpk_d[l][:, m * KH * 128:(m + 1) * KH * 128])
                    w8_tiles[(l, m)] = w[:].rearrange("p (k c) -> p k c", c=128)
                return w8_tiles[(l, m)]

            if stage >= 3:
                for m in range((P1G + NB - 1) // NB):
                    ensure_w8(2, m)

            b4s = pconst.tile([16, 1], F32, tag="b4")
            nc.sync.dma_start(b4s[:], b4_d)
            w4f = pconst.tile([128, C * KH], F16, tag="w4f")
            nc.sync.dma_start(w4f[:], w4pk_d)

            parts = {}
            stats = {}

            def st(l, tag):
                key = (l, tag)
                if key not in stats:
                    stats[key] = pstat.tile([128, KH], F32, name=f"{tag}{l}", tag=f"{tag}{l}")
                return stats[key]

            def is_fast(l):
                return l < 3 and fast[l - 1]

            def stats_chunk(l, m0, m1, g_t):
                """g_t: [128, 2d] (fast) or [128, 4d] (full): [sums | sqsums]."""
                d = m1 - m0
                red = pstat.tile([128, d], F32, tag=f"red{l}{m0}", name=f"red{l}{m0}")
                nc.vector.tensor_reduce(
                    red[:], g_t[:, 0:2 * d].rearrange("p (m n) -> p m n", n=2),
                    axis=mybir.AxisListType.X, op=ALU.add)
                if is_fast(l):
                    nc.vector.tensor_scalar_mul(st(l, "thr")[:, m0:m1], red[:], 1.0 / B)
                    return
                redq = pstat.tile([128, d], F32, tag=f"redq{l}{m0}", name=f"redq{l}{m0}")
                nc.vector.tensor_reduce(
                    redq[:], g_t[:, 2 * d:4 * d].rearrange("p (m n) -> p m n", n=2),
                    axis=mybir.AxisListType.X, op=ALU.add)
                sl = slice(m0, m1)
                m1c, msq, m1sq, v, sq, r, rp, mt, c = (
                    st(l, x) for x in ("m1", "msq", "m1sq", "v", "sq", "r", "rp", "mt", "c"))
                nc.vector.tensor_scalar_mul(m1c[:, sl], red[:], 1.0 / B)
                nc.vector.tensor_scalar_mul(msq[:, sl], redq[:], 1.0 / B)
                nc.vector.tensor_tensor(m1sq[:, sl], m1c[:, sl], m1c[:, sl], op=ALU.mult)
                nc.vector.tensor_tensor(v[:, sl], msq[:, sl], m1sq[:, sl], op=ALU.subtract)
                nc.vector.tensor_scalar_add(v[:, sl], v[:, sl], EPS)
                nc.scalar.activation(sq[:, sl], v[:, sl], ACT.Sqrt)
                nc.vector.reciprocal(r[:, sl], sq[:, sl])
                nc.vector.tensor_tensor(rp[:, sl], cons[f"g{l}"][:, sl], r[:, sl], op=ALU.mult)
                nc.vector.tensor_tensor(mt[:, sl], m1c[:, sl], rp[:, sl], op=ALU.mult)
                nc.vector.tensor_tensor(c[:, sl], cons[f"bt{l}"][:, sl], mt[:, sl], op=ALU.subtract)
                if l < 3:
                    # binarized threshold t = m - bt/(g*r); the +-sign(g) factor
                    # is folded into the next layer's weights/bias host-side
                    gi, u, u2, tthr = (st(l, x) for x in ("gi", "u", "u2", "tthr"))
                    nc.vector.reciprocal(gi[:, sl], cons[f"g{l}"][:, sl])
                    nc.vector.tensor_tensor(u[:, sl], cons[f"bt{l}"][:, sl], gi[:, sl], op=ALU.mult)
                    nc.vector.tensor_tensor(u2[:, sl], u[:, sl], sq[:, sl], op=ALU.mult)
                    nc.vector.tensor_tensor(tthr[:, sl], m1c[:, sl], u2[:, sl], op=ALU.subtract)

            def boundary(l, m0, m1, tag):
                """AllReduce parts cols for m-tiles [m0, m1) and compute stats."""
                d = m1 - m0
                w = 2 * d if is_fast(l) else 4 * d
                arin = pdram.tile([128, w], F32, tag=f"arin{l}{tag}")
                arout = pdram.tile([128, w], F32, tag=f"arout{l}{tag}")
                nc.sync.dma_start(arin[:, 0:2 * d], parts[l][:, 2 * m0:2 * m1])
                if not is_fast(l):
                    nc.sync.dma_start(arin[:, 2 * d:4 * d], parts[l][:, 32 + 2 * m0:32 + 2 * m1])
                nc.gpsimd.collective_compute(
                    "AllReduce", ALU.add, replica_groups=RG,
                    ins=[arin.opt()], outs=[arout.opt()])
                g_t = pstat.tile([128, w], F32, tag=f"g{l}{tag}", name=f"g{l}{tag}")
                nc.sync.dma_start(g_t[:], arout[:])
                stats_chunk(l, m0, m1, g_t)

            def sign_chunk(l, dst3, h_t, k0, k1, act_every=0):
                # {0,1}-coded activations: one is_ge per tile; the 2b-1
                # decode is folded into the next layer's weights (+-2) and bias
                fastl = is_fast(l)
                for k in range(k0, k1):
                    hsl = h_t[:, k * BS:(k + 1) * BS]
                    thr = st(l, "thr" if fastl else "tthr")[:, k:k + 1]
                    nc.vector.tensor_scalar(out=dst3[:, k, :], in0=hsl, scalar1=thr,
                                            scalar2=None, op0=ALU.is_ge)

            def y3_chunk(dst, h_t, k0, k1, act_every):
                rp3, c3 = st(3, "rp"), st(3, "c")
                for k in range(k0, k1):
                    hsl = h_t[:, k * BS:(k + 1) * BS]
                    scr = pscr.tile([128, BS], F32, tag="scr", name=f"y3s_{k}")
                    if act_every and (k % act_every == act_every - 1):
                        nc.scalar.activation(scr[:], hsl, ACT.Identity,
                                             bias=c3[:, k:k + 1], scale=rp3[:, k:k + 1])
                    else:
                        nc.vector.tensor_scalar(out=scr[:], in0=hsl, scalar1=rp3[:, k:k + 1],
                                                scalar2=c3[:, k:k + 1], op0=ALU.mult, op1=ALU.add)
                    nc.vector.tensor_scalar(out=dst[:, k * BS:(k + 1) * BS], in0=scr[:],
                                            scalar1=-1.0, scalar2=1.0, op0=ALU.max, op1=ALU.min)

            def finish_group(l, h_t, ps, m, n):
                hs = h_t[:, m * BS + n * 512: m * BS + n * 512 + 512]
                col = 2 * m + n
                nc.scalar.activation(hs, ps[:], ACT.Identity, bias=cons[f"b{l}"][:, m:m + 1],
                                     scale=1.0, accum_out=parts[l][:, col:col + 1])
                if not is_fast(l):
                    scr = pscr.tile([128, BS], F32, tag="scr", name=f"sq_{l}_{m}_{n}")
                    nc.scalar.activation(scr[:, :512], hs, ACT.Square,
                                         accum_out=parts[l][:, 32 + col:32 + col + 1])

            # ===================== Layer 1 =====================
            h1 = ph.tile([128, KH * BS], F32, tag="ph", name="h1")
            parts[1] = pstat.tile([128, 64], F32, tag="parts1", name="parts1")
            a2 = pa2.tile([128, KH, BS], F8E4, tag="pa2", name="a2")
            for m in range(KH):
                w16, wlov = ensure_w1(m)
                if m + 2 < KH:
                    ensure_w1(m + 2)
                for n in range(NB):
                    ps = ppsum.tile([128, 512], F32, tag="ps", name=f"ps_1_{m}_{n}")
                    if lo8:
                        for k in range(KD):
                            nc.tensor.matmul(ps[:], w16[:, k * 128:(k + 1) * 128],
                                             xhi[:, k * BS + n * 512: k * BS + n * 512 + 512],
                                             start=(k == 0), stop=False)
                        for t in range(KD // 2):
                            nc.tensor.matmul(ps[:], wlov[:, 2 * t:2 * t + 2, :],
                                             xlov[:, 2 * t:2 * t + 2, n * 512:n * 512 + 512],
                                             start=False, stop=(t == KD // 2 - 1), perf_mode=DR)
                    else:
                        for k in range(KD):
                            lhsT = w16[:, k * 128:(k + 1) * 128]
                            sl = slice(k * BS + n * 512, k * BS + n * 512 + 512)
                            nc.tensor.matmul(ps[:], lhsT, xhi[:, sl], start=(k == 0), stop=False)
                            nc.tensor.matmul(ps[:], lhsT, xlo[:, sl], start=False, stop=(k == KD - 1))
                    finish_group(1, h1, ps, m, n)
                if m == SPLIT - 1 and stage >= 2:
                    boundary(1, 0, SPLIT, "A")
                    if lo8:
                        # sign-wave A overlaps the L1 tail (a2 has its own slot)
                        sign_chunk(1, a2, h1, 0, SPLIT)
            if stage == 1:
                nc.sync.dma_start(out_d[:], h1[:C, :BS])
            if stage >= 2:
                boundary(1, SPLIT, KH, "B")
                if not lo8:
                    sign_chunk(1, a2, h1, 0, SPLIT)
                sign_chunk(1, a2, h1, SPLIT, KH)
                if stage == 2:
                    t = pscr.tile([128, BS], F32, tag="scr", name="dbg2")
                    nc.vector.tensor_scalar(out=t[:C, :], in0=a2[:C, 0, :], scalar1=2.0,
                                            scalar2=-1.0, op0=ALU.mult, op1=ALU.add)
                    nc.sync.dma_start(out_d[:], t[:C, :])

            # ================= Layers 2 and 3 (DoubleRow fp8) =================
            # stats chunk plans: early chunks fire mid-layer so the collective
            # rendezvous+exec hides under tail matmuls; the CC runs them FIFO
            CHUNK_PLAN = {2: [(0, SPLIT)], 3: [(0, 8), (8, 14)]}
            FINAL_C0 = {2: SPLIT, 3: 14}
            TSPL = {2: SPLIT // 2, 3: SPLIT // 2}  # phase-1 t-pairs
            L4P1 = 14  # L4 phase-1 k-range (y3 tiles < FINAL_C0[3])

            def dense_dr(l, rhs3, dst3_or_y3):
                h_t = ph.tile([128, KH * BS], F32, tag="ph", name=f"h{l}")
                parts[l] = pstat.tile([128, 64], F32, tag=f"parts{l}", name=f"parts{l}")
                groups = [(m, n) for m in range(KH) for n in range(NB)]
                psums = {}

                def emit(g, t0, t1):
                    m, n = groups[g]
                    wv = ensure_w8(l, m)
                    if g not in psums:
                        psums[g] = ppsum.tile([128, 512], F32, tag="ps", name=f"ps_{l}_{g}")
                    ps = psums[g]
                    for t in range(t0, t1):
                        nc.tensor.matmul(ps[:], wv[:, 2 * t:2 * t + 2, :],
                                         rhs3[:, 2 * t:2 * t + 2, n * 512:n * 512 + 512],
                                         start=(t == 0), stop=(t == KH // 2 - 1), perf_mode=DR)

                TSPLIT = TSPL[l]
                for g in range(P1G):
                    emit(g, 0, TSPLIT)
                for g in range(P1G):
                    emit(g, TSPLIT, KH // 2)
                    finish_group(l, h_t, psums[g], *groups[g])
                for g in range(P1G, len(groups)):
                    emit(g, 0, KH // 2)
                    finish_group(l, h_t, psums[g], *groups[g])
                    m, n = groups[g]
                    for c0, c1 in CHUNK_PLAN[l]:
                        if (m, n) == (c1 - 1, NB - 1):
                            boundary(l, c0, c1, f"c{c0}")
                            if l < 3:
                                sign_chunk(l, dst3_or_y3, h_t, c0, c1)
                            else:
                                # late chunks stay off the scalar queue so the
                                # tail groups' accum/Square ACTs are not delayed
                                y3_chunk(dst3_or_y3, h_t, c0, c1,
                                         act_every=1 if c0 == 0 else 0)
                    if l == 2 and stage >= 4 and (m, n) == (KH - 1, 0):
                        # all L2 weight DMAs are emitted; now prefetch L3 phase-1
                        for mp in range((P1G + NB - 1) // NB):
                            ensure_w8(3, mp)
                c0 = FINAL_C0[l]
                boundary(l, c0, KH, "Z")
                if l < 3:
                    sign_chunk(l, dst3_or_y3, h_t, c0, KH)
                else:
                    y3_chunk(dst3_or_y3, h_t, c0, KH, act_every=2)
                return h_t

            if stage >= 3:
                a3 = pa.tile([128, KH, BS], F8E4, tag="pa", name="a3")
                dense_dr(2, a2[:], a3)
                if stage == 3:
                    t = pscr.tile([128, BS], F32, tag="scr", name="dbg3")
                    nc.vector.tensor_scalar(out=t[:C, :], in0=a3[:C, 0, :], scalar1=2.0,
                                            scalar2=-1.0, op0=ALU.mult, op1=ALU.add)
                    nc.sync.dma_start(out_d[:], t[:C, :])

            if stage >= 4:
                y3 = pb.tile([128, KH * BS], F16, tag="pb", name="y3")
                dense_dr(3, a3[:], y3)
                if stage == 4:
                    t = pscr.tile([128, BS], F32, tag="scr", name="dbg4")
                    nc.vector.tensor_copy(t[:C, :], y3[:C, :BS])
                    nc.sync.dma_start(out_d[:], t[:C, :])

            if stage >= 5:
                # ===== Layer 4: logits out (log-softmax happens on the host) =====
                logits = plog.tile([16, BS], F32, tag="logits")
                ps4s = {}
                for n in range(NB):
                    ps4s[n] = ppsum.tile([128, 512], F32, tag="ps", name=f"ps4_{n}")
                    for k in range(14):
                        nc.tensor.matmul(ps4s[n][:C, :], w4f[:, k * C:(k + 1) * C],
                                         y3[:, k * BS + n * 512: k * BS + n * 512 + 512],
                                         start=(k == 0), stop=False)
                for n in range(NB):
                    for k in range(14, KH):
                        nc.tensor.matmul(ps4s[n][:C, :], w4f[:, k * C:(k + 1) * C],
                                         y3[:, k * BS + n * 512: k * BS + n * 512 + 512],
                                         start=False, stop=(k == KH - 1))
                    nc.scalar.activation(logits[:C, n * 512:(n + 1) * 512], ps4s[n][:C, :],
                                         ACT.Identity, bias=b4s[:C, :], scale=1.0)
                    nc.sync.dma_start(out_d[:, n * 512:(n + 1) * 512],
                                      logits[:C, n * 512:(n + 1) * 512])

    nc.compile()
    return nc


def _sgn(a):
    return np.where(np.asarray(a, np.float32) >= 0, np.float32(1), np.float32(-1))


def _pack(S):
    """[M*128, K*128] -> [128, M*K*128] with w[p, (m*K+k)*128+c] = S[m*128+c, k*128+p]."""
    M, K = S.shape[0] // 128, S.shape[1] // 128
    A = S.reshape(M, 128, K, 128)  # [m, c, k, p]
    return np.ascontiguousarray(A.transpose(3, 0, 2, 1).reshape(128, M * K * 128))


def _prep_inputs(x, W1, b1, g1, bt1, W2, b2, g2, bt2, W3, b3, g3, bt3, W4, b4,
                 l1mode=None):
    """Host-side sharding + layout prep (pure layout/sign/lossless-split work)."""
    l1mode = l1mode or L1MODE

    def as32(a):
        return np.ascontiguousarray(np.asarray(a, dtype=np.float32))

    x = as32(x)
    s1 = _pack(_sgn(W1))
    # {0,1}-coded activations: the device computes b = 1{h >= t}; the true
    # activation is a = s*(2b-1) with s = sign(gamma) of the producing layer.
    # Fold: h' = sum_d b_d * (2 s_d sW[f,d]) + (bias[f] - sum_d s_d sW[f,d]).
    s1mul = _sgn(g1)
    s2mul = _sgn(g2)
    S2 = _sgn(W2)
    S3 = _sgn(W3)
    shared = {
        "w1pk": s1.astype(np.float16),
        "w2pk": _pack(2.0 * S2 * s1mul[None, :]).astype(ml_dtypes.float8_e4m3fn),
        "w3pk": _pack(2.0 * S3 * s2mul[None, :]).astype(ml_dtypes.float8_e4m3fn),
    }
    if l1mode == "hi16lo8":
        shared["w1lopk"] = (s1 * 2.0 ** -11).astype(ml_dtypes.float8_e5m2)
    b2eff = as32(b2).reshape(-1) - S2 @ s1mul
    b3eff = as32(b3).reshape(-1) - S3 @ s2mul
    cvecs = (b1, g1, bt1, b2eff, g2, bt2, b3eff, g3, bt3)
    cpk = np.empty((128, KH * len(cvecs)), np.float32)
    for i, v in enumerate(cvecs):
        cpk[:, i * KH:(i + 1) * KH] = as32(v).reshape(KH, 128).T
    shared["cpk"] = cpk
    w4T = np.ascontiguousarray(as32(W4).T)          # [H, C]
    w4pk = np.empty((128, C * KH), np.float16)
    for k in range(KH):
        w4pk[:, k * C:(k + 1) * C] = w4T[k * 128:(k + 1) * 128, :].astype(np.float16)
    shared["w4pk"] = w4pk
    b4p = np.zeros((16, 1), np.float32)
    b4p[:C, 0] = as32(b4).reshape(-1)
    shared["c_b4"] = b4p

    in_maps = []
    for cidx in range(NCORES):
        xT = np.ascontiguousarray(x[cidx * BS:(cidx + 1) * BS].T)     # [D, BS]
        hi = xT.astype(np.float16)
        lo = xT - hi.astype(np.float32)
        m = dict(shared)
        m["xT_hi"] = hi
        if l1mode == "hi16lo8":
            m["xT_lo8"] = (lo * 2048.0).astype(ml_dtypes.float8_e5m2)
        else:
            m["xT_lo"] = lo.astype(np.float16)
        in_maps.append(m)
    return in_maps


def _fast_flags(inputs):
    """Mean-only BN boundaries are valid when beta==0 and gamma>0."""
    def ok(g, bt):
        g, bt = np.asarray(g), np.asarray(bt)
        return bool(not np.any(bt) and np.all(g > 0))

    return (ok(inputs["g1"], inputs["bt1"]), ok(inputs["g2"], inputs["bt2"]))


def _gather(res):
    out = np.concatenate([res.results[c]["outT"].T for c in range(NCORES)], axis=0)
    return out.astype(np.float32)


def _log_softmax(lg):
    mx = lg.max(axis=1, keepdims=True)
    return lg - mx - np.log(np.exp(lg - mx).sum(axis=1, keepdims=True))


def kernel(**inputs) -> np.ndarray:
    from concourse.bass_utils import run_bass_kernel_spmd

    fast = _fast_flags(inputs)
    if _CACHE.get("key") != (fast, L1MODE):
        _CACHE["nc"] = _build(fast=fast)
        _CACHE["key"] = (fast, L1MODE)
    nc = _CACHE["nc"]
    in_maps = _prep_inputs(**inputs)
    res = run_bass_kernel_spmd(nc, in_maps, list(range(NCORES)))
    return _log_softmax(_gather(res))


# revision 34
# speedup vs baseline: 1.0445x; 1.0343x over previous
"""Trainium2 Bass kernel for nn_BinarizedCifar10MLP.

Data-parallel over batch (8192/8 = 1024 rows/core), feature-major activation
layout [features, batch] on device.  BN batch statistics are all-reduced in two
chunks per layer so the collective latency hides under the layer's tail
matmuls; the next layer's matmuls are emitted k-phased (k < SPLIT*128 first for
the first 8 PSUM groups) so they start before the late stats chunk lands.

Precision scheme (reference is fp32, gate rel_err < 2e-2):
  - Weights are sign(+-1), pre-signed on the host: W1 as fp16 (exact), W2/W3 as
    fp8e4m3 (exact +-1) driven in DoubleRow mode at 2x PE rate.
  - L1 (x @ sign(W1).T): x split on host into fp16 hi + lo pieces.  lo either
    exact fp16 (L1MODE=hi16lo16) or fp8e5m2 of lo*2^11 matched with
    sign(W1)*2^-11 fp8e5m2 weights in DoubleRow mode (L1MODE=hi16lo8), which
    accumulates into the same fp32 PSUM group as hi.
  - L2/L3: +-1 x +-1 products accumulate exactly in fp32 PSUM.
  - L4: y3/W4 in fp16, log-softmax in fp32.
"""

import sys

sys.path.insert(0, "/opt/trn_rl_repo")

import numpy as np
import ml_dtypes

B, D, H, C = 8192, 3 * 32 * 32, 2048, 10
EPS = 1e-5
NCORES = 8
BS = B // NCORES          # batch rows per core
KD = D // 128             # 24 k-tiles over input dim
KH = H // 128             # 16 k-tiles over hidden dim
NB = BS // 512            # 2 free-dim chunks of 512
SPLIT = 12                # m-tiles covered by the early stats AllReduce chunk
P1G = 8                   # psum groups in the k-phased prologue of L2/L3
XCH = 8                   # x DMA chunks
ACT_EVERY = 4             # every ACT_EVERY-th sign/y3 tile goes to ScalarE

L1MODE = "hi16lo8"        # "hi16lo16" (exact) | "hi16lo8" (fp8 DoubleRow lo)

_CACHE = {}


def _build(stage=7, fast=(False, False), l1mode=None):
    import concourse.bacc as bacc
    import concourse.mybir as mybir
    import concourse.tile as tile

    l1mode = l1mode or L1MODE
    lo8 = l1mode == "hi16lo8"
    F32 = mybir.dt.float32
    F16 = mybir.dt.float16
    F8E4 = mybir.dt.float8e4
    F8E5 = mybir.dt.float8e5
    DR = mybir.MatmulPerfMode.DoubleRow
    ACT = mybir.ActivationFunctionType
    ALU = mybir.AluOpType
    RG = [list(range(NCORES))]

    nc = bacc.Bacc("TRN2", target_bir_lowering=False, debug=False, num_devices=NCORES)

    # ---- I/O ----
    xhi_d = nc.dram_tensor("xT_hi", [D, BS], F16, kind="ExternalInput").ap()
    if lo8:
        xlo_d = nc.dram_tensor("xT_lo8", [D, BS], F8E5, kind="ExternalInput").ap()
        w1lo_d = nc.dram_tensor("w1lopk", [128, KH * KD * 128], F8E5, kind="ExternalInput").ap()
    else:
        xlo_d = nc.dram_tensor("xT_lo", [D, BS], F16, kind="ExternalInput").ap()
        w1lo_d = None
    w1pk_d = nc.dram_tensor("w1pk", [128, KH * KD * 128], F16, kind="ExternalInput").ap()
    w2pk_d = nc.dram_tensor("w2pk", [128, KH * KH * 128], F8E4, kind="ExternalInput").ap()
    w3pk_d = nc.dram_tensor("w3pk", [128, KH * KH * 128], F8E4, kind="ExternalInput").ap()
    CNAMES = ("b1", "g1", "bt1", "b2", "g2", "bt2", "b3", "g3", "bt3")
    cpk_d = nc.dram_tensor("cpk", [128, KH * len(CNAMES)], F32, kind="ExternalInput").ap()
    w4pk_d = nc.dram_tensor("w4pk", [128, C * KH], F16, kind="ExternalInput").ap()
    b4_d = nc.dram_tensor("c_b4", [16, 1], F32, kind="ExternalInput").ap()
    out_d = nc.dram_tensor("outT", [C, BS], F32, kind="ExternalOutput").ap()
    wpk_d = {2: w2pk_d, 3: w3pk_d}

    with tile.TileContext(nc) as tc:
        with (
            tc.tile_pool(name="pconst", bufs=1) as pconst,
            tc.tile_pool(name="pstat", bufs=1) as pstat,
            tc.tile_pool(name="plog", bufs=1) as plog,
            tc.tile_pool(name="pscr", bufs=2) as pscr,
            tc.tile_pool(name="pw1", bufs=2) as pw1,
            tc.tile_pool(name="pw1lo", bufs=2) as pw1lo,
            tc.tile_pool(name="pw8", bufs=5) as pw8,
            tc.tile_pool(name="pa", bufs=1) as pa,
            tc.tile_pool(name="pb", bufs=1) as pb,
            tc.tile_pool(name="pa2", bufs=1) as pa2,
            tc.tile_pool(name="ph", bufs=1) as ph,
            tc.tile_pool(name="ppsum", bufs=8, space="PSUM") as ppsum,
            tc.tile_pool(name="pdram", bufs=1, space="DRAM") as pdram,
        ):
            # ---- L1 weight tiles (prefetch m=0,1 BEFORE x so the first
            # matmul's weights are not queued behind the whole x load) ----
            w1_tiles = {}

            def ensure_w1(m):
                if m not in w1_tiles:
                    w16 = pw1.tile([128, KD * 128], F16, tag="w1", name=f"w16_{m}")
                    nc.sync.dma_start(w16[:], w1pk_d[:, m * KD * 128:(m + 1) * KD * 128])
                    if lo8:
                        wlo = pw1lo.tile([128, KD * 128], F8E5, tag="w1lo", name=f"wlo_{m}")
                        nc.sync.dma_start(wlo[:], w1lo_d[:, m * KD * 128:(m + 1) * KD * 128])
                        w1_tiles[m] = (w16, wlo[:].rearrange("p (k c) -> p k c", c=128))
                    else:
                        w1_tiles[m] = (w16, None)
                return w1_tiles[m]

            ensure_w1(0)
            ensure_w1(1)

            # ---- load x pieces in chunks so the first matmuls start early ----
            xhi = pa.tile([128, KD * BS], F16, tag="pa", name="xhi")
            if lo8:
                xlo = pb.tile([128, KD * BS], F8E5, tag="pb", name="xlo")
            else:
                xlo = pb.tile([128, KD * BS], F16, tag="pb", name="xlo")
            xhiv = xhi[:].rearrange("p (k c) -> p k c", c=BS)
            xlov = xlo[:].rearrange("p (k c) -> p k c", c=BS)
            xhisrc = xhi_d.rearrange("(k p) c -> p k c", p=128)
            xlosrc = xlo_d.rearrange("(k p) c -> p k c", p=128)
            kpc = KD // XCH
            for ch in range(XCH):
                k0, k1 = ch * kpc, (ch + 1) * kpc
                nc.sync.dma_start(xhiv[:, k0:k1, :], xhisrc[:, k0:k1, :])
            for ch in range(XCH):
                k0, k1 = ch * kpc, (ch + 1) * kpc
                nc.sync.dma_start(xlov[:, k0:k1, :], xlosrc[:, k0:k1, :])

            # ---- constants ----
            cpk = pconst.tile([128, KH * len(CNAMES)], F32, tag="cpk")
            nc.sync.dma_start(cpk[:], cpk_d)
            cons = {name: cpk[:, i * KH:(i + 1) * KH] for i, name in enumerate(CNAMES)}

            # prefetch the L2 phase-1 weight tiles so they are resident at the
            # L1->L2 transition
            w8_tiles = {}

            def ensure_w8(l, m):
                if (l, m) not in w8_tiles:
                    w = pw8.tile([128, KH * 128], F8E4, tag="w8", name=f"w8_{l}_{m}")
                    nc.sync.dma_start(w[:], wpk_d[l][:, m * KH * 128:(m + 1) * KH * 128])
                    w8_tiles[(l, m)] = w[:].rearrange("p (k c) -> p k c", c=128)
                return w8_tiles[(l, m)]

            if stage >= 3:
                for m in range((P1G + NB - 1) // NB):
                    ensure_w8(2, m)

            b4s = pconst.tile([16, 1], F32, tag="b4")
            nc.sync.dma_start(b4s[:], b4_d)
            w4f = pconst.tile([128, C * KH], F16, tag="w4f")
            nc.sync.dma_start(w4f[:], w4pk_d)

            parts = {}
            stats = {}

            def st(l, tag):
                key = (l, tag)
                if key not in stats:
                    stats[key] = pstat.tile([128, KH], F32, name=f"{tag}{l}", tag=f"{tag}{l}")
                return stats[key]

            def is_fast(l):
                return l < 3 and fast[l - 1]

            def stats_chunk(l, m0, m1, g_t):
                """g_t: [128, 2d] (fast) or [128, 4d] (full): [sums | sqsums]."""
                d = m1 - m0
                red = pstat.tile([128, d], F32, tag=f"red{l}{m0}", name=f"red{l}{m0}")
                nc.vector.tensor_reduce(
                    red[:], g_t[:, 0:2 * d].rearrange("p (m n) -> p m n", n=2),
                    axis=mybir.AxisListType.X, op=ALU.add)
                if is_fast(l):
                    nc.vector.tensor_scalar_mul(st(l, "thr")[:, m0:m1], red[:], 1.0 / B)
                    return
                redq = pstat.tile([128, d], F32, tag=f"redq{l}{m0}", name=f"redq{l}{m0}")
                nc.vector.tensor_reduce(
                    redq[:], g_t[:, 2 * d:4 * d].rearrange("p (m n) -> p m n", n=2),
                    axis=mybir.AxisListType.X, op=ALU.add)
                sl = slice(m0, m1)
                m1c, msq, m1sq, v, sq, r, rp, mt, c = (
                    st(l, x) for x in ("m1", "msq", "m1sq", "v", "sq", "r", "rp", "mt", "c"))
                nc.vector.tensor_scalar_mul(m1c[:, sl], red[:], 1.0 / B)
                nc.vector.tensor_scalar_mul(msq[:, sl], redq[:], 1.0 / B)
                nc.vector.tensor_tensor(m1sq[:, sl], m1c[:, sl], m1c[:, sl], op=ALU.mult)
                nc.vector.tensor_tensor(v[:, sl], msq[:, sl], m1sq[:, sl], op=ALU.subtract)
                nc.vector.tensor_scalar_add(v[:, sl], v[:, sl], EPS)
                nc.scalar.activation(sq[:, sl], v[:, sl], ACT.Sqrt)
                nc.vector.reciprocal(r[:, sl], sq[:, sl])
                nc.vector.tensor_tensor(rp[:, sl], cons[f"g{l}"][:, sl], r[:, sl], op=ALU.mult)
                nc.vector.tensor_tensor(mt[:, sl], m1c[:, sl], rp[:, sl], op=ALU.mult)
                nc.vector.tensor_tensor(c[:, sl], cons[f"bt{l}"][:, sl], mt[:, sl], op=ALU.subtract)
                if l < 3:
                    # binarized threshold t = m - bt/(g*r); the +-sign(g) factor
                    # is folded into the next layer's weights/bias host-side
                    gi, u, u2, tthr = (st(l, x) for x in ("gi", "u", "u2", "tthr"))
                    nc.vector.reciprocal(gi[:, sl], cons[f"g{l}"][:, sl])
                    nc.vector.tensor_tensor(u[:, sl], cons[f"bt{l}"][:, sl], gi[:, sl], op=ALU.mult)
                    nc.vector.tensor_tensor(u2[:, sl], u[:, sl], sq[:, sl], op=ALU.mult)
                    nc.vector.tensor_tensor(tthr[:, sl], m1c[:, sl], u2[:, sl], op=ALU.subtract)

            def boundary_comm(l, m0, m1, tag):
                """AllReduce parts cols for m-tiles [m0, m1); returns the SBUF result."""
                d = m1 - m0
                w = 2 * d if is_fast(l) else 4 * d
                arin = pdram.tile([128, w], F32, tag=f"arin{l}{tag}")
                arout = pdram.tile([128, w], F32, tag=f"arout{l}{tag}")
                nc.sync.dma_start(arin[:, 0:2 * d], parts[l][:, 2 * m0:2 * m1])
                if not is_fast(l):
                    nc.sync.dma_start(arin[:, 2 * d:4 * d], parts[l][:, 32 + 2 * m0:32 + 2 * m1])
                nc.gpsimd.collective_compute(
                    "AllReduce", ALU.add, replica_groups=RG,
                    ins=[arin.opt()], outs=[arout.opt()])
                g_t = pstat.tile([128, w], F32, tag=f"g{l}{tag}", name=f"g{l}{tag}")
                nc.sync.dma_start(g_t[:], arout[:])
                return g_t

            def boundary(l, m0, m1, tag):
                stats_chunk(l, m0, m1, boundary_comm(l, m0, m1, tag))

            def sign_chunk(l, dst3, h_t, k0, k1, act_every=0):
                # {0,1}-coded activations: one is_ge per tile; the 2b-1
                # decode is folded into the next layer's weights (+-2) and bias
                fastl = is_fast(l)
                for k in range(k0, k1):
                    hsl = h_t[:, k * BS:(k + 1) * BS]
                    thr = st(l, "thr" if fastl else "tthr")[:, k:k + 1]
                    nc.vector.tensor_scalar(out=dst3[:, k, :], in0=hsl, scalar1=thr,
                                            scalar2=None, op0=ALU.is_ge)

            def y3_chunk(dst, h_t, k0, k1, act_every):
                rp3, c3 = st(3, "rp"), st(3, "c")
                for k in range(k0, k1):
                    hsl = h_t[:, k * BS:(k + 1) * BS]
                    scr = pscr.tile([128, BS], F32, tag="scr", name=f"y3s_{k}")
                    if act_every and (k % act_every == act_every - 1):
                        nc.scalar.activation(scr[:], hsl, ACT.Identity,
                                             bias=c3[:, k:k + 1], scale=rp3[:, k:k + 1])
                    else:
                        nc.vector.tensor_scalar(out=scr[:], in0=hsl, scalar1=rp3[:, k:k + 1],
                                                scalar2=c3[:, k:k + 1], op0=ALU.mult, op1=ALU.add)
                    nc.vector.tensor_scalar(out=dst[:, k * BS:(k + 1) * BS], in0=scr[:],
                                            scalar1=-1.0, scalar2=1.0, op0=ALU.max, op1=ALU.min)

            def finish_group(l, h_t, ps, m, n):
                hs = h_t[:, m * BS + n * 512: m * BS + n * 512 + 512]
                col = 2 * m + n
                nc.scalar.activation(hs, ps[:], ACT.Identity, bias=cons[f"b{l}"][:, m:m + 1],
                                     scale=1.0, accum_out=parts[l][:, col:col + 1])
                if not is_fast(l):
                    scr = pscr.tile([128, BS], F32, tag="scr", name=f"sq_{l}_{m}_{n}")
                    nc.scalar.activation(scr[:, :512], hs, ACT.Square,
                                         accum_out=parts[l][:, 32 + col:32 + col + 1])

            # ===================== Layer 1 =====================
            h1 = ph.tile([128, KH * BS], F32, tag="ph", name="h1")
            parts[1] = pstat.tile([128, 64], F32, tag="parts1", name="parts1")
            a2 = pa2.tile([128, KH, BS], F8E4, tag="pa2", name="a2")
            for m in range(KH):
                w16, wlov = ensure_w1(m)
                if m + 2 < KH:
                    ensure_w1(m + 2)
                for n in range(NB):
                    ps = ppsum.tile([128, 512], F32, tag="ps", name=f"ps_1_{m}_{n}")
                    if lo8:
                        for k in range(KD):
                            nc.tensor.matmul(ps[:], w16[:, k * 128:(k + 1) * 128],
                                             xhi[:, k * BS + n * 512: k * BS + n * 512 + 512],
                                             start=(k == 0), stop=False)
                        for t in range(KD // 2):
                            nc.tensor.matmul(ps[:], wlov[:, 2 * t:2 * t + 2, :],
                                             xlov[:, 2 * t:2 * t + 2, n * 512:n * 512 + 512],
                                             start=False, stop=(t == KD // 2 - 1), perf_mode=DR)
                    else:
                        for k in range(KD):
                            lhsT = w16[:, k * 128:(k + 1) * 128]
                            sl = slice(k * BS + n * 512, k * BS + n * 512 + 512)
                            nc.tensor.matmul(ps[:], lhsT, xhi[:, sl], start=(k == 0), stop=False)
                            nc.tensor.matmul(ps[:], lhsT, xlo[:, sl], start=False, stop=(k == KD - 1))
                    finish_group(1, h1, ps, m, n)
                if m == SPLIT - 1 and stage >= 2:
                    boundary(1, 0, SPLIT, "A")
                    if lo8:
                        # sign-wave A overlaps the L1 tail (a2 has its own slot)
                        sign_chunk(1, a2, h1, 0, SPLIT)
            if stage == 1:
                nc.sync.dma_start(out_d[:], h1[:C, :BS])
            if stage >= 2:
                boundary(1, SPLIT, KH, "B")
                if not lo8:
                    sign_chunk(1, a2, h1, 0, SPLIT)
                sign_chunk(1, a2, h1, SPLIT, KH)
                if stage == 2:
                    t = pscr.tile([128, BS], F32, tag="scr", name="dbg2")
                    nc.vector.tensor_scalar(out=t[:C, :], in0=a2[:C, 0, :], scalar1=2.0,
                                            scalar2=-1.0, op0=ALU.mult, op1=ALU.add)
                    nc.sync.dma_start(out_d[:], t[:C, :])

            # ================= Layers 2 and 3 (DoubleRow fp8) =================
            # stats chunk plans: early chunks fire mid-layer so the collective
            # rendezvous+exec hides under tail matmuls; the CC runs them FIFO
            CHUNK_PLAN = {2: [(0, SPLIT)], 3: [(0, 8), (8, 14)]}
            FINAL_C0 = {2: SPLIT, 3: 14}
            TSPL = {2: SPLIT // 2, 3: SPLIT // 2}  # phase-1 t-pairs
            L4P1 = 14  # L4 phase-1 k-range (y3 tiles < FINAL_C0[3])

            def dense_dr(l, rhs3, dst3_or_y3):
                h_t = ph.tile([128, KH * BS], F32, tag="ph", name=f"h{l}")
                parts[l] = pstat.tile([128, 64], F32, tag=f"parts{l}", name=f"parts{l}")
                groups = [(m, n) for m in range(KH) for n in range(NB)]
                psums = {}

                def emit(g, t0, t1):
                    m, n = groups[g]
                    wv = ensure_w8(l, m)
                    if g not in psums:
                        psums[g] = ppsum.tile([128, 512], F32, tag="ps", name=f"ps_{l}_{g}")
                    ps = psums[g]
                    for t in range(t0, t1):
                        nc.tensor.matmul(ps[:], wv[:, 2 * t:2 * t + 2, :],
                                         rhs3[:, 2 * t:2 * t + 2, n * 512:n * 512 + 512],
                                         start=(t == 0), stop=(t == KH // 2 - 1), perf_mode=DR)

                TSPLIT = TSPL[l]
                for g in range(P1G):
                    emit(g, 0, TSPLIT)
                for g in range(P1G):
                    emit(g, TSPLIT, KH // 2)
                    finish_group(l, h_t, psums[g], *groups[g])
                deferred = []
                for g in range(P1G, len(groups)):
                    emit(g, 0, KH // 2)
                    finish_group(l, h_t, psums[g], *groups[g])
                    m, n = groups[g]
                    for c0, c1 in CHUNK_PLAN[l]:
                        if (m, n) == (c1 - 1, NB - 1):
                            if l < 3:
                                boundary(l, c0, c1, f"c{c0}")
                                sign_chunk(l, dst3_or_y3, h_t, c0, c1)
                            else:
                                # l=3: only launch the collective here; stats/y3
                                # contain scalar-queue ops gated on the AllReduce
                                # which would head-of-line-stall the accum ACTs
                                deferred.append(
                                    (c0, c1, boundary_comm(l, c0, c1, f"c{c0}")))
                    if l == 2 and stage >= 4 and (m, n) == (KH - 1, 0):
                        # all L2 weight DMAs are emitted; now prefetch L3 phase-1
                        for mp in range((P1G + NB - 1) // NB):
                            ensure_w8(3, mp)
                c0 = FINAL_C0[l]
                if l < 3:
                    boundary(l, c0, KH, "Z")
                    sign_chunk(l, dst3_or_y3, h_t, c0, KH)
                else:
                    deferred.append((c0, KH, boundary_comm(l, c0, KH, "Z")))
                    for d0, d1, g_t in deferred:
                        stats_chunk(l, d0, d1, g_t)
                        y3_chunk(dst3_or_y3, h_t, d0, d1, act_every=0)
                return h_t

            if stage >= 3:
                a3 = pa.tile([128, KH, BS], F8E4, tag="pa", name="a3")
                dense_dr(2, a2[:], a3)
                if stage == 3:
                    t = pscr.tile([128, BS], F32, tag="scr", name="dbg3")
                    nc.vector.tensor_scalar(out=t[:C, :], in0=a3[:C, 0, :], scalar1=2.0,
                                            scalar2=-1.0, op0=ALU.mult, op1=ALU.add)
                    nc.sync.dma_start(out_d[:], t[:C, :])

            if stage >= 4:
                y3 = pb.tile([128, KH * BS], F16, tag="pb", name="y3")
                dense_dr(3, a3[:], y3)
                if stage == 4:
                    t = pscr.tile([128, BS], F32, tag="scr", name="dbg4")
                    nc.vector.tensor_copy(t[:C, :], y3[:C, :BS])
                    nc.sync.dma_start(out_d[:], t[:C, :])

            if stage >= 5:
                # ===== Layer 4: logits out (log-softmax happens on the host) =====
                logits = plog.tile([16, BS], F32, tag="logits")
                ps4s = {}
                for n in range(NB):
                    ps4s[n] = ppsum.tile([128, 512], F32, tag="ps", name=f"ps4_{n}")
                    for k in range(14):
                        nc.tensor.matmul(ps4s[n][:C, :], w4f[:, k * C:(k + 1) * C],
                                         y3[:, k * BS + n * 512: k * BS + n * 512 + 512],
                                         start=(k == 0), stop=False)
                for n in range(NB):
                    for k in range(14, KH):
                        nc.tensor.matmul(ps4s[n][:C, :], w4f[:, k * C:(k + 1) * C],
                                         y3[:, k * BS + n * 512: k * BS + n * 512 + 512],
                                         start=False, stop=(k == KH - 1))
                    nc.scalar.activation(logits[:C, n * 512:(n + 1) * 512], ps4s[n][:C, :],
                                         ACT.Identity, bias=b4s[:C, :], scale=1.0)
                    nc.sync.dma_start(out_d[:, n * 512:(n + 1) * 512],
                                      logits[:C, n * 512:(n + 1) * 512])

    nc.compile()
    return nc


def _sgn(a):
    return np.where(np.asarray(a, np.float32) >= 0, np.float32(1), np.float32(-1))


def _pack(S):
    """[M*128, K*128] -> [128, M*K*128] with w[p, (m*K+k)*128+c] = S[m*128+c, k*128+p]."""
    M, K = S.shape[0] // 128, S.shape[1] // 128
    A = S.reshape(M, 128, K, 128)  # [m, c, k, p]
    return np.ascontiguousarray(A.transpose(3, 0, 2, 1).reshape(128, M * K * 128))


def _prep_inputs(x, W1, b1, g1, bt1, W2, b2, g2, bt2, W3, b3, g3, bt3, W4, b4,
                 l1mode=None):
    """Host-side sharding + layout prep (pure layout/sign/lossless-split work)."""
    l1mode = l1mode or L1MODE

    def as32(a):
        return np.ascontiguousarray(np.asarray(a, dtype=np.float32))

    x = as32(x)
    s1 = _pack(_sgn(W1))
    # {0,1}-coded activations: the device computes b = 1{h >= t}; the true
    # activation is a = s*(2b-1) with s = sign(gamma) of the producing layer.
    # Fold: h' = sum_d b_d * (2 s_d sW[f,d]) + (bias[f] - sum_d s_d sW[f,d]).
    s1mul = _sgn(g1)
    s2mul = _sgn(g2)
    S2 = _sgn(W2)
    S3 = _sgn(W3)
    shared = {
        "w1pk": s1.astype(np.float16),
        "w2pk": _pack(2.0 * S2 * s1mul[None, :]).astype(ml_dtypes.float8_e4m3fn),
        "w3pk": _pack(2.0 * S3 * s2mul[None, :]).astype(ml_dtypes.float8_e4m3fn),
    }
    if l1mode == "hi16lo8":
        shared["w1lopk"] = (s1 * 2.0 ** -11).astype(ml_dtypes.float8_e5m2)
    b2eff = as32(b2).reshape(-1) - S2 @ s1mul
    b3eff = as32(b3).reshape(-1) - S3 @ s2mul
    cvecs = (b1, g1, bt1, b2eff, g2, bt2, b3eff, g3, bt3)
    cpk = np.empty((128, KH * len(cvecs)), np.float32)
    for i, v in enumerate(cvecs):
        cpk[:, i * KH:(i + 1) * KH] = as32(v).reshape(KH, 128).T
    shared["cpk"] = cpk
    w4T = np.ascontiguousarray(as32(W4).T)          # [H, C]
    w4pk = np.empty((128, C * KH), np.float16)
    for k in range(KH):
        w4pk[:, k * C:(k + 1) * C] = w4T[k * 128:(k + 1) * 128, :].astype(np.float16)
    shared["w4pk"] = w4pk
    b4p = np.zeros((16, 1), np.float32)
    b4p[:C, 0] = as32(b4).reshape(-1)
    shared["c_b4"] = b4p

    in_maps = []
    for cidx in range(NCORES):
        xT = np.ascontiguousarray(x[cidx * BS:(cidx + 1) * BS].T)     # [D, BS]
        hi = xT.astype(np.float16)
        lo = xT - hi.astype(np.float32)
        m = dict(shared)
        m["xT_hi"] = hi
        if l1mode == "hi16lo8":
            m["xT_lo8"] = (lo * 2048.0).astype(ml_dtypes.float8_e5m2)
        else:
            m["xT_lo"] = lo.astype(np.float16)
        in_maps.append(m)
    return in_maps


def _fast_flags(inputs):
    """Mean-only BN boundaries are valid when beta==0 and gamma>0."""
    def ok(g, bt):
        g, bt = np.asarray(g), np.asarray(bt)
        return bool(not np.any(bt) and np.all(g > 0))

    return (ok(inputs["g1"], inputs["bt1"]), ok(inputs["g2"], inputs["bt2"]))


def _gather(res):
    out = np.concatenate([res.results[c]["outT"].T for c in range(NCORES)], axis=0)
    return out.astype(np.float32)


def _log_softmax(lg):
    mx = lg.max(axis=1, keepdims=True)
    return lg - mx - np.log(np.exp(lg - mx).sum(axis=1, keepdims=True))


def kernel(**inputs) -> np.ndarray:
    from concourse.bass_utils import run_bass_kernel_spmd

    fast = _fast_flags(inputs)
    if _CACHE.get("key") != (fast, L1MODE):
        _CACHE["nc"] = _build(fast=fast)
        _CACHE["key"] = (fast, L1MODE)
    nc = _CACHE["nc"]
    in_maps = _prep_inputs(**inputs)
    res = run_bass_kernel_spmd(nc, in_maps, list(range(NCORES)))
    return _log_softmax(_gather(res))


# revision 36
# speedup vs baseline: 1.0658x; 1.0203x over previous
"""Trainium2 Bass kernel for nn_BinarizedCifar10MLP.

Data-parallel over batch (8192/8 = 1024 rows/core), feature-major activation
layout [features, batch] on device.  BN batch statistics are all-reduced in two
chunks per layer so the collective latency hides under the layer's tail
matmuls; the next layer's matmuls are emitted k-phased (k < SPLIT*128 first for
the first 8 PSUM groups) so they start before the late stats chunk lands.

Precision scheme (reference is fp32, gate rel_err < 2e-2):
  - Weights are sign(+-1), pre-signed on the host: W1 as fp16 (exact), W2/W3 as
    fp8e4m3 (exact +-1) driven in DoubleRow mode at 2x PE rate.
  - L1 (x @ sign(W1).T): x split on host into fp16 hi + lo pieces.  lo either
    exact fp16 (L1MODE=hi16lo16) or fp8e5m2 of lo*2^11 matched with
    sign(W1)*2^-11 fp8e5m2 weights in DoubleRow mode (L1MODE=hi16lo8), which
    accumulates into the same fp32 PSUM group as hi.
  - L2/L3: +-1 x +-1 products accumulate exactly in fp32 PSUM.
  - L4: y3/W4 in fp16, log-softmax in fp32.
"""

import sys

sys.path.insert(0, "/opt/trn_rl_repo")

import numpy as np
import ml_dtypes

B, D, H, C = 8192, 3 * 32 * 32, 2048, 10
EPS = 1e-5
NCORES = 8
BS = B // NCORES          # batch rows per core
KD = D // 128             # 24 k-tiles over input dim
KH = H // 128             # 16 k-tiles over hidden dim
NB = BS // 512            # 2 free-dim chunks of 512
SPLIT = 12                # m-tiles covered by the early stats AllReduce chunk
P1G = 8                   # psum groups in the k-phased prologue of L2/L3
XCH = 8                   # x DMA chunks
ACT_EVERY = 4             # every ACT_EVERY-th sign/y3 tile goes to ScalarE

L1MODE = "hi16lo8"        # "hi16lo16" (exact) | "hi16lo8" (fp8 DoubleRow lo)

_CACHE = {}


def _build(stage=7, fast=(False, False), l1mode=None):
    import concourse.bacc as bacc
    import concourse.mybir as mybir
    import concourse.tile as tile

    l1mode = l1mode or L1MODE
    lo8 = l1mode == "hi16lo8"
    F32 = mybir.dt.float32
    F16 = mybir.dt.float16
    F8E4 = mybir.dt.float8e4
    F8E5 = mybir.dt.float8e5
    DR = mybir.MatmulPerfMode.DoubleRow
    ACT = mybir.ActivationFunctionType
    ALU = mybir.AluOpType
    RG = [list(range(NCORES))]

    nc = bacc.Bacc("TRN2", target_bir_lowering=False, debug=False, num_devices=NCORES)

    # ---- I/O ----
    xhi_d = nc.dram_tensor("xT_hi", [D, BS], F16, kind="ExternalInput").ap()
    if lo8:
        xlo_d = nc.dram_tensor("xT_lo8", [D, BS], F8E5, kind="ExternalInput").ap()
        w1lo_d = nc.dram_tensor("w1lopk", [128, KH * KD * 128], F8E5, kind="ExternalInput").ap()
    else:
        xlo_d = nc.dram_tensor("xT_lo", [D, BS], F16, kind="ExternalInput").ap()
        w1lo_d = None
    w1pk_d = nc.dram_tensor("w1pk", [128, KH * KD * 128], F16, kind="ExternalInput").ap()
    w2pk_d = nc.dram_tensor("w2pk", [128, KH * KH * 128], F8E4, kind="ExternalInput").ap()
    w3pk_d = nc.dram_tensor("w3pk", [128, KH * KH * 128], F8E4, kind="ExternalInput").ap()
    CNAMES = ("b1", "g1", "bt1", "b2", "g2", "bt2", "b3", "g3", "bt3")
    cpk_d = nc.dram_tensor("cpk", [128, KH * len(CNAMES)], F32, kind="ExternalInput").ap()
    w4pk_d = nc.dram_tensor("w4pk", [128, C * KH], F16, kind="ExternalInput").ap()
    b4_d = nc.dram_tensor("c_b4", [16, 1], F32, kind="ExternalInput").ap()
    out_d = nc.dram_tensor("outT", [C, BS], F32, kind="ExternalOutput").ap()
    wpk_d = {2: w2pk_d, 3: w3pk_d}

    with tile.TileContext(nc) as tc:
        with (
            tc.tile_pool(name="pconst", bufs=1) as pconst,
            tc.tile_pool(name="pstat", bufs=1) as pstat,
            tc.tile_pool(name="plog", bufs=1) as plog,
            tc.tile_pool(name="pscr", bufs=2) as pscr,
            tc.tile_pool(name="pw1", bufs=2) as pw1,
            tc.tile_pool(name="pw1lo", bufs=2) as pw1lo,
            tc.tile_pool(name="pw8", bufs=5) as pw8,
            tc.tile_pool(name="pa", bufs=1) as pa,
            tc.tile_pool(name="pb", bufs=1) as pb,
            tc.tile_pool(name="pa2", bufs=1) as pa2,
            tc.tile_pool(name="ph", bufs=1) as ph,
            tc.tile_pool(name="ppsum", bufs=8, space="PSUM") as ppsum,
            tc.tile_pool(name="pdram", bufs=1, space="DRAM") as pdram,
        ):
            # ---- L1 weight tiles (prefetch m=0,1 BEFORE x so the first
            # matmul's weights are not queued behind the whole x load) ----
            w1_tiles = {}

            def ensure_w1(m):
                if m not in w1_tiles:
                    w16 = pw1.tile([128, KD * 128], F16, tag="w1", name=f"w16_{m}")
                    nc.sync.dma_start(w16[:], w1pk_d[:, m * KD * 128:(m + 1) * KD * 128])
                    if lo8:
                        wlo = pw1lo.tile([128, KD * 128], F8E5, tag="w1lo", name=f"wlo_{m}")
                        nc.sync.dma_start(wlo[:], w1lo_d[:, m * KD * 128:(m + 1) * KD * 128])
                        w1_tiles[m] = (w16, wlo[:].rearrange("p (k c) -> p k c", c=128))
                    else:
                        w1_tiles[m] = (w16, None)
                return w1_tiles[m]

            ensure_w1(0)
            ensure_w1(1)

            # ---- load x pieces in chunks so the first matmuls start early ----
            xhi = pa.tile([128, KD * BS], F16, tag="pa", name="xhi")
            if lo8:
                xlo = pb.tile([128, KD * BS], F8E5, tag="pb", name="xlo")
            else:
                xlo = pb.tile([128, KD * BS], F16, tag="pb", name="xlo")
            xhiv = xhi[:].rearrange("p (k c) -> p k c", c=BS)
            xlov = xlo[:].rearrange("p (k c) -> p k c", c=BS)
            xhisrc = xhi_d.rearrange("(k p) c -> p k c", p=128)
            xlosrc = xlo_d.rearrange("(k p) c -> p k c", p=128)
            kpc = KD // XCH
            for ch in range(XCH):
                k0, k1 = ch * kpc, (ch + 1) * kpc
                nc.sync.dma_start(xhiv[:, k0:k1, :], xhisrc[:, k0:k1, :])
            for ch in range(XCH):
                k0, k1 = ch * kpc, (ch + 1) * kpc
                nc.sync.dma_start(xlov[:, k0:k1, :], xlosrc[:, k0:k1, :])

            # ---- constants ----
            cpk = pconst.tile([128, KH * len(CNAMES)], F32, tag="cpk")
            nc.sync.dma_start(cpk[:], cpk_d)
            cons = {name: cpk[:, i * KH:(i + 1) * KH] for i, name in enumerate(CNAMES)}

            # prefetch the L2 phase-1 weight tiles so they are resident at the
            # L1->L2 transition
            w8_tiles = {}

            def ensure_w8(l, m):
                if (l, m) not in w8_tiles:
                    w = pw8.tile([128, KH * 128], F8E4, tag="w8", name=f"w8_{l}_{m}")
                    nc.sync.dma_start(w[:], wpk_d[l][:, m * KH * 128:(m + 1) * KH * 128])
                    w8_tiles[(l, m)] = w[:].rearrange("p (k c) -> p k c", c=128)
                return w8_tiles[(l, m)]

            if stage >= 3:
                for m in range((P1G + NB - 1) // NB):
                    ensure_w8(2, m)

            b4s = pconst.tile([16, 1], F32, tag="b4")
            nc.sync.dma_start(b4s[:], b4_d)
            w4f = pconst.tile([128, C * KH], F16, tag="w4f")
            nc.sync.dma_start(w4f[:], w4pk_d)

            parts = {}
            stats = {}

            def st(l, tag):
                key = (l, tag)
                if key not in stats:
                    stats[key] = pstat.tile([128, KH], F32, name=f"{tag}{l}", tag=f"{tag}{l}")
                return stats[key]

            def is_fast(l):
                return l < 3 and fast[l - 1]

            def stats_chunk(l, m0, m1, g_t):
                """g_t: [128, 2d] (fast) or [128, 4d] (full): [sums | sqsums]."""
                d = m1 - m0
                red = pstat.tile([128, d], F32, tag=f"red{l}{m0}", name=f"red{l}{m0}")
                nc.vector.tensor_reduce(
                    red[:], g_t[:, 0:2 * d].rearrange("p (m n) -> p m n", n=2),
                    axis=mybir.AxisListType.X, op=ALU.add)
                if is_fast(l):
                    nc.vector.tensor_scalar_mul(st(l, "thr")[:, m0:m1], red[:], 1.0 / B)
                    return
                redq = pstat.tile([128, d], F32, tag=f"redq{l}{m0}", name=f"redq{l}{m0}")
                nc.vector.tensor_reduce(
                    redq[:], g_t[:, 2 * d:4 * d].rearrange("p (m n) -> p m n", n=2),
                    axis=mybir.AxisListType.X, op=ALU.add)
                sl = slice(m0, m1)
                m1c, msq, m1sq, v, sq, r, rp, mt, c = (
                    st(l, x) for x in ("m1", "msq", "m1sq", "v", "sq", "r", "rp", "mt", "c"))
                nc.vector.tensor_scalar_mul(m1c[:, sl], red[:], 1.0 / B)
                nc.vector.tensor_scalar_mul(msq[:, sl], redq[:], 1.0 / B)
                nc.vector.tensor_tensor(m1sq[:, sl], m1c[:, sl], m1c[:, sl], op=ALU.mult)
                nc.vector.tensor_tensor(v[:, sl], msq[:, sl], m1sq[:, sl], op=ALU.subtract)
                nc.vector.tensor_scalar_add(v[:, sl], v[:, sl], EPS)
                nc.scalar.activation(sq[:, sl], v[:, sl], ACT.Sqrt)
                nc.vector.reciprocal(r[:, sl], sq[:, sl])
                nc.vector.tensor_tensor(rp[:, sl], cons[f"g{l}"][:, sl], r[:, sl], op=ALU.mult)
                nc.vector.tensor_tensor(mt[:, sl], m1c[:, sl], rp[:, sl], op=ALU.mult)
                nc.vector.tensor_tensor(c[:, sl], cons[f"bt{l}"][:, sl], mt[:, sl], op=ALU.subtract)
                if l < 3:
                    # binarized threshold t = m - bt/(g*r); the +-sign(g) factor
                    # is folded into the next layer's weights/bias host-side
                    gi, u, u2, tthr = (st(l, x) for x in ("gi", "u", "u2", "tthr"))
                    nc.vector.reciprocal(gi[:, sl], cons[f"g{l}"][:, sl])
                    nc.vector.tensor_tensor(u[:, sl], cons[f"bt{l}"][:, sl], gi[:, sl], op=ALU.mult)
                    nc.vector.tensor_tensor(u2[:, sl], u[:, sl], sq[:, sl], op=ALU.mult)
                    nc.vector.tensor_tensor(tthr[:, sl], m1c[:, sl], u2[:, sl], op=ALU.subtract)

            def boundary_comm(l, m0, m1, tag):
                """AllReduce parts cols for m-tiles [m0, m1); returns the SBUF result."""
                d = m1 - m0
                w = 2 * d if is_fast(l) else 4 * d
                arin = pdram.tile([128, w], F32, tag=f"arin{l}{tag}")
                arout = pdram.tile([128, w], F32, tag=f"arout{l}{tag}")
                nc.sync.dma_start(arin[:, 0:2 * d], parts[l][:, 2 * m0:2 * m1])
                if not is_fast(l):
                    nc.sync.dma_start(arin[:, 2 * d:4 * d], parts[l][:, 32 + 2 * m0:32 + 2 * m1])
                nc.gpsimd.collective_compute(
                    "AllReduce", ALU.add, replica_groups=RG,
                    ins=[arin.opt()], outs=[arout.opt()])
                g_t = pstat.tile([128, w], F32, tag=f"g{l}{tag}", name=f"g{l}{tag}")
                nc.sync.dma_start(g_t[:], arout[:])
                return g_t

            def boundary(l, m0, m1, tag):
                stats_chunk(l, m0, m1, boundary_comm(l, m0, m1, tag))

            def sign_chunk(l, dst3, h_t, k0, k1, act_every=0):
                # {0,1}-coded activations: one is_ge per tile; the 2b-1
                # decode is folded into the next layer's weights (+-2) and bias
                fastl = is_fast(l)
                for k in range(k0, k1):
                    hsl = h_t[:, k * BS:(k + 1) * BS]
                    thr = st(l, "thr" if fastl else "tthr")[:, k:k + 1]
                    nc.vector.tensor_scalar(out=dst3[:, k, :], in0=hsl, scalar1=thr,
                                            scalar2=None, op0=ALU.is_ge)

            def y3_chunk(dst, h_t, k0, k1, act_every):
                # act_every > 0: every act_every-th tile on ScalarE;
                # act_every < 0: all EXCEPT every |act_every|-th tile on ScalarE
                rp3, c3 = st(3, "rp"), st(3, "c")
                for k in range(k0, k1):
                    hsl = h_t[:, k * BS:(k + 1) * BS]
                    scr = pscr.tile([128, BS], F32, tag="scr", name=f"y3s_{k}")
                    use_act = (act_every > 0 and k % act_every == act_every - 1) or \
                              (act_every < 0 and k % (-act_every) != (-act_every) - 1)
                    if use_act:
                        nc.scalar.activation(scr[:], hsl, ACT.Identity,
                                             bias=c3[:, k:k + 1], scale=rp3[:, k:k + 1])
                    else:
                        nc.vector.tensor_scalar(out=scr[:], in0=hsl, scalar1=rp3[:, k:k + 1],
                                                scalar2=c3[:, k:k + 1], op0=ALU.mult, op1=ALU.add)
                    nc.vector.tensor_scalar(out=dst[:, k * BS:(k + 1) * BS], in0=scr[:],
                                            scalar1=-1.0, scalar2=1.0, op0=ALU.max, op1=ALU.min)

            def finish_group(l, h_t, ps, m, n):
                hs = h_t[:, m * BS + n * 512: m * BS + n * 512 + 512]
                col = 2 * m + n
                nc.scalar.activation(hs, ps[:], ACT.Identity, bias=cons[f"b{l}"][:, m:m + 1],
                                     scale=1.0, accum_out=parts[l][:, col:col + 1])
                if not is_fast(l):
                    scr = pscr.tile([128, BS], F32, tag="scr", name=f"sq_{l}_{m}_{n}")
                    nc.scalar.activation(scr[:, :512], hs, ACT.Square,
                                         accum_out=parts[l][:, 32 + col:32 + col + 1])

            # ===================== Layer 1 =====================
            h1 = ph.tile([128, KH * BS], F32, tag="ph", name="h1")
            parts[1] = pstat.tile([128, 64], F32, tag="parts1", name="parts1")
            a2 = pa2.tile([128, KH, BS], F8E4, tag="pa2", name="a2")
            for m in range(KH):
                w16, wlov = ensure_w1(m)
                if m + 2 < KH:
                    ensure_w1(m + 2)
                for n in range(NB):
                    ps = ppsum.tile([128, 512], F32, tag="ps", name=f"ps_1_{m}_{n}")
                    if lo8:
                        for k in range(KD):
                            nc.tensor.matmul(ps[:], w16[:, k * 128:(k + 1) * 128],
                                             xhi[:, k * BS + n * 512: k * BS + n * 512 + 512],
                                             start=(k == 0), stop=False)
                        for t in range(KD // 2):
                            nc.tensor.matmul(ps[:], wlov[:, 2 * t:2 * t + 2, :],
                                             xlov[:, 2 * t:2 * t + 2, n * 512:n * 512 + 512],
                                             start=False, stop=(t == KD // 2 - 1), perf_mode=DR)
                    else:
                        for k in range(KD):
                            lhsT = w16[:, k * 128:(k + 1) * 128]
                            sl = slice(k * BS + n * 512, k * BS + n * 512 + 512)
                            nc.tensor.matmul(ps[:], lhsT, xhi[:, sl], start=(k == 0), stop=False)
                            nc.tensor.matmul(ps[:], lhsT, xlo[:, sl], start=False, stop=(k == KD - 1))
                    finish_group(1, h1, ps, m, n)
                if m == SPLIT - 1 and stage >= 2:
                    boundary(1, 0, SPLIT, "A")
                    if lo8:
                        # sign-wave A overlaps the L1 tail (a2 has its own slot)
                        sign_chunk(1, a2, h1, 0, SPLIT)
            if stage == 1:
                nc.sync.dma_start(out_d[:], h1[:C, :BS])
            if stage >= 2:
                boundary(1, SPLIT, KH, "B")
                if not lo8:
                    sign_chunk(1, a2, h1, 0, SPLIT)
                sign_chunk(1, a2, h1, SPLIT, KH)
                if stage == 2:
                    t = pscr.tile([128, BS], F32, tag="scr", name="dbg2")
                    nc.vector.tensor_scalar(out=t[:C, :], in0=a2[:C, 0, :], scalar1=2.0,
                                            scalar2=-1.0, op0=ALU.mult, op1=ALU.add)
                    nc.sync.dma_start(out_d[:], t[:C, :])

            # ================= Layers 2 and 3 (DoubleRow fp8) =================
            # stats chunk plans: early chunks fire mid-layer so the collective
            # rendezvous+exec hides under tail matmuls; the CC runs them FIFO
            CHUNK_PLAN = {2: [(0, SPLIT)], 3: [(0, 8), (8, 14)]}
            FINAL_C0 = {2: SPLIT, 3: 14}
            TSPL = {2: SPLIT // 2, 3: SPLIT // 2}  # phase-1 t-pairs
            L4P1 = 14  # L4 phase-1 k-range (y3 tiles < FINAL_C0[3])

            def dense_dr(l, rhs3, dst3_or_y3):
                h_t = ph.tile([128, KH * BS], F32, tag="ph", name=f"h{l}")
                parts[l] = pstat.tile([128, 64], F32, tag=f"parts{l}", name=f"parts{l}")
                groups = [(m, n) for m in range(KH) for n in range(NB)]
                psums = {}

                def emit(g, t0, t1):
                    m, n = groups[g]
                    wv = ensure_w8(l, m)
                    if g not in psums:
                        psums[g] = ppsum.tile([128, 512], F32, tag="ps", name=f"ps_{l}_{g}")
                    ps = psums[g]
                    for t in range(t0, t1):
                        nc.tensor.matmul(ps[:], wv[:, 2 * t:2 * t + 2, :],
                                         rhs3[:, 2 * t:2 * t + 2, n * 512:n * 512 + 512],
                                         start=(t == 0), stop=(t == KH // 2 - 1), perf_mode=DR)

                TSPLIT = TSPL[l]
                for g in range(P1G):
                    emit(g, 0, TSPLIT)
                for g in range(P1G):
                    emit(g, TSPLIT, KH // 2)
                    finish_group(l, h_t, psums[g], *groups[g])
                deferred = []
                for g in range(P1G, len(groups)):
                    emit(g, 0, KH // 2)
                    finish_group(l, h_t, psums[g], *groups[g])
                    m, n = groups[g]
                    for c0, c1 in CHUNK_PLAN[l]:
                        if (m, n) == (c1 - 1, NB - 1):
                            if l < 3:
                                boundary(l, c0, c1, f"c{c0}")
                                sign_chunk(l, dst3_or_y3, h_t, c0, c1)
                            else:
                                # l=3: only launch the collective here; stats/y3
                                # contain scalar-queue ops gated on the AllReduce
                                # which would head-of-line-stall the accum ACTs
                                deferred.append(
                                    (c0, c1, boundary_comm(l, c0, c1, f"c{c0}")))
                    if l == 2 and stage >= 4 and (m, n) == (KH - 1, 0):
                        # all L2 weight DMAs are emitted; now prefetch L3 phase-1
                        for mp in range((P1G + NB - 1) // NB):
                            ensure_w8(3, mp)
                c0 = FINAL_C0[l]
                if l < 3:
                    boundary(l, c0, KH, "Z")
                    sign_chunk(l, dst3_or_y3, h_t, c0, KH)
                else:
                    deferred.append((c0, KH, boundary_comm(l, c0, KH, "Z")))
                    for d0, d1, g_t in deferred:
                        stats_chunk(l, d0, d1, g_t)
                        # scalar queue is clear of accum ACTs here; split y3
                        # ~3:1 ScalarE:DVE to halve the chain latency
                        y3_chunk(dst3_or_y3, h_t, d0, d1, act_every=-4)
                return h_t

            if stage >= 3:
                a3 = pa.tile([128, KH, BS], F8E4, tag="pa", name="a3")
                dense_dr(2, a2[:], a3)
                if stage == 3:
                    t = pscr.tile([128, BS], F32, tag="scr", name="dbg3")
                    nc.vector.tensor_scalar(out=t[:C, :], in0=a3[:C, 0, :], scalar1=2.0,
                                            scalar2=-1.0, op0=ALU.mult, op1=ALU.add)
                    nc.sync.dma_start(out_d[:], t[:C, :])

            if stage >= 4:
                y3 = pb.tile([128, KH * BS], F16, tag="pb", name="y3")
                dense_dr(3, a3[:], y3)
                if stage == 4:
                    t = pscr.tile([128, BS], F32, tag="scr", name="dbg4")
                    nc.vector.tensor_copy(t[:C, :], y3[:C, :BS])
                    nc.sync.dma_start(out_d[:], t[:C, :])

            if stage >= 5:
                # ===== Layer 4: logits out (log-softmax happens on the host) =====
                logits = plog.tile([16, BS], F32, tag="logits")
                ps4s = {}
                for n in range(NB):
                    ps4s[n] = ppsum.tile([128, 512], F32, tag="ps", name=f"ps4_{n}")
                    for k in range(14):
                        nc.tensor.matmul(ps4s[n][:C, :], w4f[:, k * C:(k + 1) * C],
                                         y3[:, k * BS + n * 512: k * BS + n * 512 + 512],
                                         start=(k == 0), stop=False)
                for n in range(NB):
                    for k in range(14, KH):
                        nc.tensor.matmul(ps4s[n][:C, :], w4f[:, k * C:(k + 1) * C],
                                         y3[:, k * BS + n * 512: k * BS + n * 512 + 512],
                                         start=False, stop=(k == KH - 1))
                    nc.scalar.activation(logits[:C, n * 512:(n + 1) * 512], ps4s[n][:C, :],
                                         ACT.Identity, bias=b4s[:C, :], scale=1.0)
                    nc.sync.dma_start(out_d[:, n * 512:(n + 1) * 512],
                                      logits[:C, n * 512:(n + 1) * 512])

    nc.compile()
    return nc


def _sgn(a):
    return np.where(np.asarray(a, np.float32) >= 0, np.float32(1), np.float32(-1))


def _pack(S):
    """[M*128, K*128] -> [128, M*K*128] with w[p, (m*K+k)*128+c] = S[m*128+c, k*128+p]."""
    M, K = S.shape[0] // 128, S.shape[1] // 128
    A = S.reshape(M, 128, K, 128)  # [m, c, k, p]
    return np.ascontiguousarray(A.transpose(3, 0, 2, 1).reshape(128, M * K * 128))


def _prep_inputs(x, W1, b1, g1, bt1, W2, b2, g2, bt2, W3, b3, g3, bt3, W4, b4,
                 l1mode=None):
    """Host-side sharding + layout prep (pure layout/sign/lossless-split work)."""
    l1mode = l1mode or L1MODE

    def as32(a):
        return np.ascontiguousarray(np.asarray(a, dtype=np.float32))

    x = as32(x)
    s1 = _pack(_sgn(W1))
    # {0,1}-coded activations: the device computes b = 1{h >= t}; the true
    # activation is a = s*(2b-1) with s = sign(gamma) of the producing layer.
    # Fold: h' = sum_d b_d * (2 s_d sW[f,d]) + (bias[f] - sum_d s_d sW[f,d]).
    s1mul = _sgn(g1)
    s2mul = _sgn(g2)
    S2 = _sgn(W2)
    S3 = _sgn(W3)
    shared = {
        "w1pk": s1.astype(np.float16),
        "w2pk": _pack(2.0 * S2 * s1mul[None, :]).astype(ml_dtypes.float8_e4m3fn),
        "w3pk": _pack(2.0 * S3 * s2mul[None, :]).astype(ml_dtypes.float8_e4m3fn),
    }
    if l1mode == "hi16lo8":
        shared["w1lopk"] = (s1 * 2.0 ** -11).astype(ml_dtypes.float8_e5m2)
    b2eff = as32(b2).reshape(-1) - S2 @ s1mul
    b3eff = as32(b3).reshape(-1) - S3 @ s2mul
    cvecs = (b1, g1, bt1, b2eff, g2, bt2, b3eff, g3, bt3)
    cpk = np.empty((128, KH * len(cvecs)), np.float32)
    for i, v in enumerate(cvecs):
        cpk[:, i * KH:(i + 1) * KH] = as32(v).reshape(KH, 128).T
    shared["cpk"] = cpk
    w4T = np.ascontiguousarray(as32(W4).T)          # [H, C]
    w4pk = np.empty((128, C * KH), np.float16)
    for k in range(KH):
        w4pk[:, k * C:(k + 1) * C] = w4T[k * 128:(k + 1) * 128, :].astype(np.float16)
    shared["w4pk"] = w4pk
    b4p = np.zeros((16, 1), np.float32)
    b4p[:C, 0] = as32(b4).reshape(-1)
    shared["c_b4"] = b4p

    in_maps = []
    for cidx in range(NCORES):
        xT = np.ascontiguousarray(x[cidx * BS:(cidx + 1) * BS].T)     # [D, BS]
        hi = xT.astype(np.float16)
        lo = xT - hi.astype(np.float32)
        m = dict(shared)
        m["xT_hi"] = hi
        if l1mode == "hi16lo8":
            m["xT_lo8"] = (lo * 2048.0).astype(ml_dtypes.float8_e5m2)
        else:
            m["xT_lo"] = lo.astype(np.float16)
        in_maps.append(m)
    return in_maps


def _fast_flags(inputs):
    """Mean-only BN boundaries are valid when beta==0 and gamma>0."""
    def ok(g, bt):
        g, bt = np.asarray(g), np.asarray(bt)
        return bool(not np.any(bt) and np.all(g > 0))

    return (ok(inputs["g1"], inputs["bt1"]), ok(inputs["g2"], inputs["bt2"]))


def _gather(res):
    out = np.concatenate([res.results[c]["outT"].T for c in range(NCORES)], axis=0)
    return out.astype(np.float32)


def _log_softmax(lg):
    mx = lg.max(axis=1, keepdims=True)
    return lg - mx - np.log(np.exp(lg - mx).sum(axis=1, keepdims=True))


def kernel(**inputs) -> np.ndarray:
    from concourse.bass_utils import run_bass_kernel_spmd

    fast = _fast_flags(inputs)
    if _CACHE.get("key") != (fast, L1MODE):
        _CACHE["nc"] = _build(fast=fast)
        _CACHE["key"] = (fast, L1MODE)
    nc = _CACHE["nc"]
    in_maps = _prep_inputs(**inputs)
    res = run_bass_kernel_spmd(nc, in_maps, list(range(NCORES)))
    return _log_softmax(_gather(res))
